# revision 15
# baseline (speedup 1.0000x reference)
"""Trainium2 Bass kernel for local-window Bahdanau attention.

Problem (hardcoded shapes): B=1024, L=100 (10x10 grid), C=1024, U=512,
window 3x3 (D=1).  Reference computes:
    p_t   = sigmoid(tanh(h @ Wa) @ Wb) * 8 + 1          (B,1,2)
    st    = int32(p_t - 1) clamped to [0, 7]            (B,2)
    local = grid[b, st0:st0+3, st1:st1+3, :]            (B,9,C)
    score = tanh(local @ W1 + W1_b + h @ W2 + W2_b)     (B,9,U)
    attn  = softmax(score @ V1 + V1_b, axis=1) * gauss  (B,9,1)
    ctx   = sum(attn * local, axis=1)                   (B,C)
    out   = tanh(concat([ctx, h])) @ W3 + W3_b          (B,U)
returns (out, attn).

Strategy: pure data-parallel over 8 NeuronCores (128 examples each).
The 3x3 window rows are fetched with dma_gather (device-side indices
computed from hidden), so only 9/100 of `features` is ever read.
Matmul-heavy parts run in bf16 (fp32 PSUM accumulation); the index
computation path (p_t) is kept entirely in fp32 to bit-match the
reference's truncation.
"""

import sys

if "/opt/trn_rl_repo" not in sys.path:
    sys.path.insert(0, "/opt/trn_rl_repo")

import numpy as np
import ml_dtypes

import concourse.bass as bass
import concourse.bacc as bacc
import concourse.mybir as mybir
from concourse.ap import AP
from concourse.tile import TileContext, add_dep_helper
from concourse.bass_utils import run_bass_kernel_spmd
from concourse.library_config import mlp

F32 = mybir.dt.float32
BF16 = mybir.dt.bfloat16
I16 = mybir.dt.int16
I32 = mybir.dt.int32
AF = mybir.ActivationFunctionType
ALU = mybir.AluOpType

B, L, C, U = 1024, 100, 1024, 512
G, WIN, D = 10, 3, 1
NL = WIN * WIN            # 9 window positions
NC_CORES = 8
BS = B // NC_CORES        # 128 examples per core
NROWS = BS * L            # 12800 feature rows per core
KC = C // 128             # 8 contraction chunks over C
KU = U // 128             # 4 chunks over U
KT = (C + U) // 128       # 12 chunks over C+U
RPW = NL * BS             # 1152 score rows per core


def _build_nc():
    nc = bacc.Bacc("TRN2", target_bir_lowering=False)

    feat = nc.dram_tensor("feat", [NROWS, C], F32, kind="ExternalInput")
    hid = nc.dram_tensor("hid", [BS, U], F32, kind="ExternalInput")
    w1 = nc.dram_tensor("w1", [C, U], BF16, kind="ExternalInput")
    w2 = nc.dram_tensor("w2", [U, U], BF16, kind="ExternalInput")
    w3 = nc.dram_tensor("w3", [C + U, U], BF16, kind="ExternalInput")
    wa = nc.dram_tensor("wa", [U, 100], F32, kind="ExternalInput")
    wb = nc.dram_tensor("wb", [100, 2], F32, kind="ExternalInput")
    v1 = nc.dram_tensor("v1", [U, 1], BF16, kind="ExternalInput")
    b12 = nc.dram_tensor("b12", [U, 1], F32, kind="ExternalInput")
    w3b = nc.dram_tensor("w3b", [128, U], F32, kind="ExternalInput")
    iota = nc.dram_tensor("iota", [16, 8], F32, kind="ExternalInput")
    eye = nc.dram_tensor("eye", [128, 128], BF16, kind="ExternalInput")
    gauss = nc.dram_tensor("gauss", [128, NL], F32, kind="ExternalInput")
    id32 = nc.dram_tensor("id32", [128, 128], F32, kind="ExternalInput")
    comb2 = nc.dram_tensor("comb2", [2, 16], F32, kind="ExternalInput")
    mask16 = nc.dram_tensor("mask16", [16, 128], F32, kind="ExternalInput")

    out = nc.dram_tensor("out", [BS, U], F32, kind="ExternalOutput")
    attn = nc.dram_tensor("attn", [BS, NL], F32, kind="ExternalOutput")

    with TileContext(nc) as tc:
        lib_inst = nc.gpsimd.load_library(mlp)

        with tc.tile_pool(name="sb", bufs=1) as sb:
            # ---- persistent SBUF tiles ----
            h_nat = sb.tile([BS, U], F32, tag="h_nat")
            hT32 = sb.tile([128, U], F32, tag="hT32")
            hT16 = sb.tile([128, U], BF16, tag="hT16")
            th16 = sb.tile([128, U], BF16, tag="th16")
            w1_sb = sb.tile([128, KC * U], BF16, tag="w1_sb")
            w2_sb = sb.tile([128, KU * U], BF16, tag="w2_sb")
            w3_sb = sb.tile([128, KT * U], BF16, tag="w3_sb")
            wa_sb = sb.tile([128, KU * 100], F32, tag="wa_sb")
            wb_sb = sb.tile([100, 2], F32, tag="wb_sb")
            v1_sb = sb.tile([128, KU], BF16, tag="v1_sb")
            b12_sb = sb.tile([128, KU], F32, tag="b12_sb")
            w3b_sb = sb.tile([128, U], F32, tag="w3b_sb")
            iota_sb = sb.tile([16, 8], F32, tag="iota_sb")
            eye_sb = sb.tile([128, 128], BF16, tag="eye_sb")
            gauss_sb = sb.tile([128, NL], F32, tag="gauss_sb")
            id_sb = sb.tile([128, 128], F32, tag="id_sb")
            comb2_sb = sb.tile([2, 16], F32, tag="comb2_sb")
            mask_sb = sb.tile([16, 128], F32, tag="mask_sb")

            local_nat = sb.tile([128, NL * C], F32, tag="local_nat")
            local_bf = sb.tile([128, NL * C], BF16, tag="local_bf")
            localT = sb.tile([128, KC * RPW], BF16, tag="localT")
            w2h_sb = sb.tile([128, KU * 128], F32, tag="w2h_sb")
            scoreT = sb.tile([128, KU * RPW], BF16, tag="scoreT")
            tct16 = sb.tile([128, KC * 128], BF16, tag="tct16")
            diag = sb.tile([128, NL * 128], BF16, tag="diag")
            out_sb = sb.tile([BS, U], F32, tag="out_sb")

            t1 = sb.tile([128, 128], F32, tag="t1")
            t2 = sb.tile([2, 128], F32, tag="t2")
            pm1 = sb.tile([2, 128], F32, tag="pm1")
            ci = sb.tile([2, 128], I32, tag="ci")
            cf = sb.tile([2, 128], F32, tag="cf")
            gt = sb.tile([2, 128], F32, tag="gt")
            st = sb.tile([2, 128], F32, tag="st")
            s16m = sb.tile([16, 128], F32, tag="s16m")
            base16 = sb.tile([16, 8], F32, tag="base16")
            idxf = sb.tile([16, 24], F32, tag="idxf")
            idx16 = sb.tile([16, 24], I16, tag="idx16")
            idxrep = sb.tile([128, 24], I16, tag="idxrep")
            stmp = sb.tile([128, RPW], F32, tag="stmp")
            negmax = sb.tile([128, 1], F32, tag="negmax")
            esum = sb.tile([128, 1], F32, tag="esum")
            rsum = sb.tile([128, 1], F32, tag="rsum")
            e_sb = sb.tile([128, NL], F32, tag="e_sb")
            attn_f = sb.tile([128, NL], F32, tag="attn_f")

            # ---- input DMAs (critical-path items first) ----
            nc.sync.dma_start(h_nat[:], hid[:])
            nc.sync.dma_start(wa_sb[:].rearrange("p (k u) -> p k u", k=KU),
                              wa.rearrange("(k p) u -> p k u", p=128))
            nc.sync.dma_start(wb_sb[:], wb[:])
            nc.sync.dma_start(id_sb[:], id32[:])
            nc.sync.dma_start(comb2_sb[:], comb2[:])
            nc.sync.dma_start(mask_sb[:], mask16[:])
            nc.sync.dma_start(iota_sb[:], iota[:])
            nc.sync.dma_start(w2_sb[:].rearrange("p (k u) -> p k u", k=KU),
                              w2.rearrange("(k p) u -> p k u", p=128))
            nc.sync.dma_start(b12_sb[:].unsqueeze(2),
                              b12.rearrange("(k p) o -> p k o", p=128))
            nc.sync.dma_start(w1_sb[:].rearrange("p (k u) -> p k u", k=KC),
                              w1.rearrange("(k p) u -> p k u", p=128))
            nc.sync.dma_start(v1_sb[:].unsqueeze(2),
                              v1.rearrange("(k p) o -> p k o", p=128))
            nc.sync.dma_start(eye_sb[:], eye[:])
            nc.sync.dma_start(gauss_sb[:], gauss[:])
            nc.sync.dma_start(w3_sb[:].rearrange("p (k u) -> p k u", k=KT),
                              w3.rearrange("(k p) u -> p k u", p=128))
            nc.sync.dma_start(w3b_sb[:], w3b[:])

            # ---- hT (PE transpose, fp32) ----
            with tc.tile_pool(name="pmps", bufs=2, space="PSUM") as pmps:
                for k in range(KU):
                    ps = pmps.tile([128, 128], F32, tag="ps_tr")
                    nc.tensor.transpose(ps[:], h_nat[:, k * 128:(k + 1) * 128], id_sb[:])
                    nc.vector.tensor_copy(hT32[:, k * 128:(k + 1) * 128], ps[:])
                nc.vector.tensor_copy(hT16[:], hT32[:])
                nc.scalar.activation(th16[:], hT32[:], AF.Tanh)

                # ---- p_t chain (fp32 end to end) ----
                z1 = pmps.tile([128, 128], F32, tag="ps_z")
                for k in range(KU):
                    nc.tensor.matmul(z1[0:100, :], wa_sb[:, k * 100:(k + 1) * 100],
                                     hT32[:, k * 128:(k + 1) * 128],
                                     start=(k == 0), stop=(k == KU - 1))
                nc.scalar.activation(t1[0:100, :], z1[0:100, :], AF.Tanh)
                z2 = pmps.tile([128, 128], F32, tag="ps_z")
                nc.tensor.matmul(z2[0:2, :], wb_sb[:], t1[0:100, :], start=True, stop=True)
                # p_t - 1 = 8*sigmoid(z) = 4*tanh(z/2) + 4
                nc.scalar.activation(t2[:], z2[0:2, :], AF.Tanh, scale=0.5)
                nc.vector.tensor_scalar(pm1[:], t2[:], 4.0, 4.0, ALU.mult, ALU.add)
                # floor (rounding-mode agnostic): c=int(x); c -= (c > x)
                nc.vector.tensor_copy(ci[:], pm1[:])
                nc.vector.tensor_copy(cf[:], ci[:])
                nc.vector.tensor_tensor(gt[:], cf[:], pm1[:], ALU.is_gt)
                nc.vector.tensor_tensor(st[:], cf[:], gt[:], ALU.subtract)
                nc.vector.tensor_scalar(st[:], st[:], 0.0, float(G - WIN), ALU.max, ALU.min)
                # s16[m, b] = 10*st0[b] + st1[b], broadcast to 16 partitions
                s16 = pmps.tile([16, 128], F32, tag="ps_z")
                nc.tensor.matmul(s16[:], comb2_sb[:], st[:], start=True, stop=True)
                # per-partition diagonal extract: base16[p, q] = s16[p, q*16+p]
                nc.vector.tensor_tensor(s16m[:], s16[:], mask_sb[:], ALU.mult)
                nc.vector.reduce_sum(base16[:],
                                     s16m[:].rearrange("p (q i) -> p q i", i=16),
                                     axis=mybir.AxisListType.X)
                nc.vector.tensor_tensor(base16[:], base16[:], iota_sb[:], ALU.add)
                for j in range(WIN):
                    nc.vector.tensor_scalar_add(idxf[:, j * 8:(j + 1) * 8], base16[:],
                                                float(G * j))
                nc.vector.tensor_copy(idx16[:], idxf[:])
                for r in range(8):
                    nc.sync.dma_start(idxrep[16 * r:16 * (r + 1), :], idx16[:])

                # ---- w2h = (h @ W2)^T  [uo*128+m, b] ----
                for uo in range(KU):
                    psw = pmps.tile([128, 128], F32, tag="ps_w2h")
                    for ui in range(KU):
                        nc.tensor.matmul(
                            psw[:],
                            w2_sb[:, ui * U + uo * 128: ui * U + (uo + 1) * 128],
                            hT16[:, ui * 128:(ui + 1) * 128],
                            start=(ui == 0), stop=(ui == KU - 1))
                    nc.vector.tensor_copy(w2h_sb[:, uo * 128:(uo + 1) * 128], psw[:])

            # ---- gather the 3x3 windows (3 gathers: one grid-row each) ----
            feat_gap = AP(feat.ap().tensor, 0, [[C, NROWS - 2], [1, WIN * C]])
            for j in range(WIN):
                g = nc.gpsimd.dma_gather(
                    local_nat[:, j * WIN * C:(j + 1) * WIN * C]
                        .rearrange("p (o e) -> p o e", o=1),
                    feat_gap,
                    idxrep[:, j * 8:(j + 1) * 8],
                    BS, BS, WIN * C,
                    elem_step=C,
                )
                add_dep_helper(g.ins, lib_inst.ins, reason="load_library before gather")

            # cast to bf16 + xbar-transpose into [c, row] layout
            for j in range(WIN):
                nc.vector.tensor_copy(local_bf[:, j * WIN * C:(j + 1) * WIN * C],
                                      local_nat[:, j * WIN * C:(j + 1) * WIN * C])
                for li in range(WIN):
                    l = j * WIN + li
                    for cc in range(KC):
                        nc.sync.dma_start(
                            localT[:, cc * RPW + l * 128:cc * RPW + (l + 1) * 128],
                            local_bf[:, l * C + cc * 128:l * C + (cc + 1) * 128],
                            transpose=True)

            # ---- scoreT = tanh(W1^T localT + w2h + b12)  [u, l*128+b] ----
            with tc.tile_pool(name="sps", bufs=2, space="PSUM") as sps:
                for uo in range(KU):
                    pss = [sps.tile([128, 384], F32, tag=f"ps_s{j}",
                                    name=f"pss{uo}_{j}")
                           for j in range(WIN)]
                    for j in range(WIN):
                        for k in range(KC):
                            nc.tensor.matmul(
                                pss[j][:],
                                w1_sb[:, k * U + uo * 128:k * U + (uo + 1) * 128],
                                localT[:, k * RPW + j * 384:k * RPW + (j + 1) * 384],
                                start=(k == 0), stop=(k == KC - 1))
                        nc.vector.tensor_tensor(
                            stmp[:, j * 384:(j + 1) * 384]
                                .rearrange("p (l b) -> p l b", b=128),
                            pss[j][:].rearrange("p (l b) -> p l b", b=128),
                            w2h_sb[:, uo * 128:(uo + 1) * 128].unsqueeze(1)
                                .broadcast_to([128, WIN, 128]),
                            ALU.add)
                    nc.scalar.activation(scoreT[:, uo * RPW:(uo + 1) * RPW], stmp[:],
                                         AF.Tanh, bias=b12_sb[:, uo:uo + 1])

            # ---- logits -> softmax -> attn ----
            with tc.tile_pool(name="lgps", bufs=1, space="PSUM") as lgps:
                lg = lgps.tile([128, NL], F32, tag="ps_lg")
                for l in range(NL):
                    for uo in range(KU):
                        nc.tensor.matmul(
                            lg[:, l:l + 1],
                            scoreT[:, uo * RPW + l * 128:uo * RPW + (l + 1) * 128],
                            v1_sb[:, uo:uo + 1],
                            start=(uo == 0), stop=(uo == KU - 1))
                nc.vector.tensor_reduce(negmax[:], lg[:], axis=mybir.AxisListType.X,
                                        op=ALU.max, negate=True)
                nc.scalar.activation(e_sb[:], lg[:], AF.Exp, bias=negmax[:])
            nc.vector.reduce_sum(esum[:], e_sb[:], axis=mybir.AxisListType.X)
            nc.vector.reciprocal(rsum[:], esum[:])
            nc.vector.scalar_tensor_tensor(attn_f[:], e_sb[:], rsum[:], gauss_sb[:],
                                           ALU.mult, ALU.mult)
            nc.sync.dma_start(attn[:], attn_f[:])

            # ---- ctx^T via diag matmuls, then tanh -> tct16 ----
            for l in range(NL):
                nc.vector.tensor_scalar_mul(diag[:, l * 128:(l + 1) * 128], eye_sb[:],
                                            attn_f[:, l:l + 1])
            with tc.tile_pool(name="cps", bufs=4, space="PSUM") as cps, \
                 tc.tile_pool(name="ops", bufs=1, space="PSUM") as ops:
                for cc in range(KC):
                    pc = cps.tile([128, 128], F32, tag="ps_c")
                    for l in range(NL):
                        nc.tensor.matmul(pc[:],
                                         local_bf[:, l * C + cc * 128:l * C + (cc + 1) * 128],
                                         diag[:, l * 128:(l + 1) * 128],
                                         start=(l == 0), stop=(l == NL - 1))
                    nc.scalar.activation(tct16[:, cc * 128:(cc + 1) * 128], pc[:], AF.Tanh)

                # ---- out = tanh([ctx, h]) @ W3 + W3_b ----
                po = ops.tile([128, U], F32, tag="ps_o")
                for kk in range(KT):
                    lhsT = (tct16[:, kk * 128:(kk + 1) * 128] if kk < KC
                            else th16[:, (kk - KC) * 128:(kk - KC + 1) * 128])
                    nc.tensor.matmul(po[:], lhsT, w3_sb[:, kk * U:(kk + 1) * U],
                                     start=(kk == 0), stop=(kk == KT - 1))
                nc.vector.tensor_tensor(out_sb[:], po[:], w3b_sb[:], ALU.add)
            nc.sync.dma_start(out[:], out_sb[:])

    nc.compile()
    return nc


_NC_CACHE = None


def _get_nc():
    global _NC_CACHE
    if _NC_CACHE is None:
        _NC_CACHE = _build_nc()
    return _NC_CACHE


def make_host_inputs(features, hidden, W1_w, W1_b, W2_w, W2_b, V1_w, V1_b,
                     W3_w, W3_b, Wa, Wb):
    """Build the 8 per-core input maps."""
    bf = ml_dtypes.bfloat16
    f = np.float32

    jj, kk = np.meshgrid(np.arange(WIN), np.arange(WIN), indexing="ij")
    d2 = (jj - WIN / 2.0) ** 2 + (kk - WIN / 2.0) ** 2
    gauss_row = np.exp(-d2 / (0.5 * D * D)).reshape(NL).astype(f)

    p = np.arange(16)
    q = np.arange(8)
    iota = (L * (q[None, :] * 16 + p[:, None])).astype(f)

    shared = {
        "w1": np.ascontiguousarray(W1_w, f).astype(bf),
        "w2": np.ascontiguousarray(W2_w, f).astype(bf),
        "w3": np.ascontiguousarray(W3_w, f).astype(bf),
        "wa": np.ascontiguousarray(Wa, f),
        "wb": np.ascontiguousarray(Wb, f),
        "v1": np.ascontiguousarray(V1_w, f).astype(bf),
        "b12": (np.asarray(W1_b, f) + np.asarray(W2_b, f)).reshape(U, 1),
        "w3b": np.broadcast_to(np.asarray(W3_b, f), (128, U)).copy(),
        "iota": iota,
        "eye": np.eye(128, dtype=f).astype(bf),
        "gauss": np.broadcast_to(gauss_row, (128, NL)).copy(),
        "id32": np.eye(128, dtype=f),
        "comb2": np.stack([np.full(16, float(G), f), np.ones(16, f)]),
        "mask16": (np.arange(128)[None, :] % 16 == np.arange(16)[:, None])
                  .astype(f),
    }
    features = np.asarray(features, f)
    hidden = np.asarray(hidden, f)
    in_maps = []
    for c in range(NC_CORES):
        sl = slice(c * BS, (c + 1) * BS)
        m = dict(shared)
        m["feat"] = np.ascontiguousarray(features[sl]).reshape(NROWS, C)
        m["hid"] = np.ascontiguousarray(hidden[sl])
        in_maps.append(m)
    return in_maps


def kernel(features, hidden, W1_w, W1_b, W2_w, W2_b, V1_w, V1_b,
           W3_w, W3_b, Wa, Wb, _run_kwargs=None):
    nc = _get_nc()
    in_maps = make_host_inputs(features, hidden, W1_w, W1_b, W2_w, W2_b,
                               V1_w, V1_b, W3_w, W3_b, Wa, Wb)
    res = run_bass_kernel_spmd(nc, in_maps, core_ids=list(range(NC_CORES)),
                               **(_run_kwargs or {}))
    out = np.concatenate([r["out"] for r in res.results], axis=0)
    attn = np.concatenate([r["attn"] for r in res.results], axis=0)
    kernel.last_results = res
    return out, attn.reshape(B, NL, 1)


# revision 18
# speedup vs baseline: 1.7957x; 1.7957x over previous
"""Trainium2 Bass kernel for local-window Bahdanau attention.

Problem (hardcoded shapes): B=1024, L=100 (10x10 grid), C=1024, U=512,
window 3x3 (D=1).  Reference computes:
    p_t   = sigmoid(tanh(h @ Wa) @ Wb) * 8 + 1          (B,1,2)
    st    = int32(p_t - 1) clamped to [0, 7]            (B,2)
    local = grid[b, st0:st0+3, st1:st1+3, :]            (B,9,C)
    score = tanh(local @ W1 + W1_b + h @ W2 + W2_b)     (B,9,U)
    attn  = softmax(score @ V1 + V1_b, axis=1) * gauss  (B,9,1)
    ctx   = sum(attn * local, axis=1)                   (B,C)
    out   = tanh(concat([ctx, h])) @ W3 + W3_b          (B,U)
returns (out, attn).

Strategy: pure data-parallel over 8 NeuronCores (128 examples each).
The 3x3 window rows are fetched with dma_gather (device-side indices
computed from hidden), so only 9/100 of `features` is ever read.
Matmul-heavy parts run in bf16 (fp32 PSUM accumulation); the index
computation path (p_t) is kept entirely in fp32 to bit-match the
reference's truncation.
"""

import sys

if "/opt/trn_rl_repo" not in sys.path:
    sys.path.insert(0, "/opt/trn_rl_repo")

import numpy as np
import ml_dtypes

import concourse.bass as bass
import concourse.bacc as bacc
import concourse.mybir as mybir
from concourse.ap import AP
from concourse.tile import TileContext, add_dep_helper
from concourse.bass_utils import run_bass_kernel_spmd
from concourse.library_config import mlp

F32 = mybir.dt.float32
BF16 = mybir.dt.bfloat16
I16 = mybir.dt.int16
I32 = mybir.dt.int32
AF = mybir.ActivationFunctionType
ALU = mybir.AluOpType

B, L, C, U = 1024, 100, 1024, 512
G, WIN, D = 10, 3, 1
NL = WIN * WIN            # 9 window positions
NC_CORES = 8
BS = B // NC_CORES        # 128 examples per core
NROWS = BS * L            # 12800 feature rows per core
KC = C // 128             # 8 contraction chunks over C
KU = U // 128             # 4 chunks over U
KT = (C + U) // 128       # 12 chunks over C+U
RPW = NL * BS             # 1152 score rows per core


def _build_nc():
    nc = bacc.Bacc("TRN2", target_bir_lowering=False)

    feat = nc.dram_tensor("feat", [NROWS, C], F32, kind="ExternalInput")
    hid = nc.dram_tensor("hid", [BS, U], F32, kind="ExternalInput")
    w1 = nc.dram_tensor("w1", [C, U], BF16, kind="ExternalInput")
    w2 = nc.dram_tensor("w2", [U, U], BF16, kind="ExternalInput")
    w3 = nc.dram_tensor("w3", [C + U, U], BF16, kind="ExternalInput")
    wa = nc.dram_tensor("wa", [U, 100], F32, kind="ExternalInput")
    wb = nc.dram_tensor("wb", [100, 2], F32, kind="ExternalInput")
    v1 = nc.dram_tensor("v1", [U, 1], BF16, kind="ExternalInput")
    b12 = nc.dram_tensor("b12", [U, 1], F32, kind="ExternalInput")
    w3b = nc.dram_tensor("w3b", [128, U], F32, kind="ExternalInput")
    iota = nc.dram_tensor("iota", [16, 8], F32, kind="ExternalInput")
    eye = nc.dram_tensor("eye", [128, 128], BF16, kind="ExternalInput")
    gauss = nc.dram_tensor("gauss", [128, NL], F32, kind="ExternalInput")
    id32 = nc.dram_tensor("id32", [128, 128], F32, kind="ExternalInput")
    comb2 = nc.dram_tensor("comb2", [2, 16], F32, kind="ExternalInput")
    mask16 = nc.dram_tensor("mask16", [16, 128], F32, kind="ExternalInput")

    out = nc.dram_tensor("out", [BS, U], F32, kind="ExternalOutput")
    attn = nc.dram_tensor("attn", [BS, NL], F32, kind="ExternalOutput")

    with TileContext(nc) as tc:
        lib_inst = nc.gpsimd.load_library(mlp)

        with tc.tile_pool(name="sb", bufs=1) as sb:
            # ---- persistent SBUF tiles ----
            h_nat = sb.tile([BS, U], F32, tag="h_nat")
            hT32 = sb.tile([128, U], F32, tag="hT32")
            hT16 = sb.tile([128, U], BF16, tag="hT16")
            th16 = sb.tile([128, U], BF16, tag="th16")
            w1_sb = sb.tile([128, KC * U], BF16, tag="w1_sb")
            w2_sb = sb.tile([128, KU * U], BF16, tag="w2_sb")
            w3_sb = sb.tile([128, KT * U], BF16, tag="w3_sb")
            wa_sb = sb.tile([128, KU * 100], F32, tag="wa_sb")
            wb_sb = sb.tile([100, 2], F32, tag="wb_sb")
            v1_sb = sb.tile([128, KU], BF16, tag="v1_sb")
            b12_sb = sb.tile([128, KU], F32, tag="b12_sb")
            w3b_sb = sb.tile([128, U], F32, tag="w3b_sb")
            iota_sb = sb.tile([16, 8], F32, tag="iota_sb")
            eye_sb = sb.tile([128, 128], BF16, tag="eye_sb")
            gauss_sb = sb.tile([128, NL], F32, tag="gauss_sb")
            id_sb = sb.tile([128, 128], F32, tag="id_sb")
            comb2_sb = sb.tile([2, 16], F32, tag="comb2_sb")
            mask_sb = sb.tile([16, 128], F32, tag="mask_sb")

            local_nat = sb.tile([128, NL * C], F32, tag="local_nat")
            local_bf = sb.tile([128, NL * C], BF16, tag="local_bf")
            localT = sb.tile([128, KC * RPW], BF16, tag="localT")
            w2h_sb = sb.tile([128, KU * 128], F32, tag="w2h_sb")
            scoreT = sb.tile([128, KU * RPW], BF16, tag="scoreT")
            tct16 = sb.tile([128, KC * 128], BF16, tag="tct16")
            diag = sb.tile([128, NL * 128], BF16, tag="diag")
            out_sb = sb.tile([BS, U], F32, tag="out_sb")

            t1 = sb.tile([128, 128], F32, tag="t1")
            t2 = sb.tile([2, 128], F32, tag="t2")
            pm1 = sb.tile([2, 128], F32, tag="pm1")
            ci = sb.tile([2, 128], I32, tag="ci")
            cf = sb.tile([2, 128], F32, tag="cf")
            gt = sb.tile([2, 128], F32, tag="gt")
            st = sb.tile([2, 128], F32, tag="st")
            s16m = sb.tile([16, 128], F32, tag="s16m")
            base16 = sb.tile([16, 8], F32, tag="base16")
            idxf = sb.tile([16, 24], F32, tag="idxf")
            idx16 = sb.tile([16, 24], I16, tag="idx16")
            idxrep = sb.tile([128, 24], I16, tag="idxrep")
            stmp = sb.tile([128, RPW], F32, tag="stmp")
            negmax = sb.tile([128, 1], F32, tag="negmax")
            esum = sb.tile([128, 1], F32, tag="esum")
            rsum = sb.tile([128, 1], F32, tag="rsum")
            e_sb = sb.tile([128, NL], F32, tag="e_sb")
            attn_f = sb.tile([128, NL], F32, tag="attn_f")

            # ---- input DMAs (critical-path items first) ----
            nc.sync.dma_start(h_nat[:], hid[:])
            nc.sync.dma_start(wa_sb[:].rearrange("p (k u) -> p k u", k=KU),
                              wa.rearrange("(k p) u -> p k u", p=128))
            nc.sync.dma_start(wb_sb[:], wb[:])
            nc.sync.dma_start(id_sb[:], id32[:])
            nc.sync.dma_start(comb2_sb[:], comb2[:])
            nc.sync.dma_start(mask_sb[:], mask16[:])
            nc.sync.dma_start(iota_sb[:], iota[:])
            nc.sync.dma_start(w2_sb[:].rearrange("p (k u) -> p k u", k=KU),
                              w2.rearrange("(k p) u -> p k u", p=128))
            nc.sync.dma_start(b12_sb[:].unsqueeze(2),
                              b12.rearrange("(k p) o -> p k o", p=128))
            nc.scalar.dma_start(w1_sb[:].rearrange("p (k u) -> p k u", k=KC),
                              w1.rearrange("(k p) u -> p k u", p=128))
            nc.scalar.dma_start(v1_sb[:].unsqueeze(2),
                              v1.rearrange("(k p) o -> p k o", p=128))
            nc.scalar.dma_start(eye_sb[:], eye[:])
            nc.scalar.dma_start(gauss_sb[:], gauss[:])
            nc.scalar.dma_start(w3_sb[:].rearrange("p (k u) -> p k u", k=KT),
                              w3.rearrange("(k p) u -> p k u", p=128))
            nc.scalar.dma_start(w3b_sb[:], w3b[:])

            # ---- hT (PE transpose, fp32) ----
            with tc.tile_pool(name="pmps", bufs=2, space="PSUM") as pmps:
                for k in range(KU):
                    ps = pmps.tile([128, 128], F32, tag="ps_tr")
                    nc.tensor.transpose(ps[:], h_nat[:, k * 128:(k + 1) * 128], id_sb[:])
                    nc.vector.tensor_copy(hT32[:, k * 128:(k + 1) * 128], ps[:])
                nc.vector.tensor_copy(hT16[:], hT32[:])
                nc.scalar.activation(th16[:], hT32[:], AF.Tanh)

                # ---- p_t chain (fp32 end to end) ----
                z1 = pmps.tile([128, 128], F32, tag="ps_z")
                for k in range(KU):
                    nc.tensor.matmul(z1[0:100, :], wa_sb[:, k * 100:(k + 1) * 100],
                                     hT32[:, k * 128:(k + 1) * 128],
                                     start=(k == 0), stop=(k == KU - 1))
                nc.scalar.activation(t1[0:100, :], z1[0:100, :], AF.Tanh)
                z2 = pmps.tile([128, 128], F32, tag="ps_z")
                nc.tensor.matmul(z2[0:2, :], wb_sb[:], t1[0:100, :], start=True, stop=True)
                # p_t - 1 = 8*sigmoid(z) = 4*tanh(z/2) + 4
                nc.scalar.activation(t2[:], z2[0:2, :], AF.Tanh, scale=0.5)
                nc.vector.tensor_scalar(pm1[:], t2[:], 4.0, 4.0, ALU.mult, ALU.add)
                # floor (rounding-mode agnostic): c=int(x); c -= (c > x)
                nc.vector.tensor_copy(ci[:], pm1[:])
                nc.vector.tensor_copy(cf[:], ci[:])
                nc.vector.tensor_tensor(gt[:], cf[:], pm1[:], ALU.is_gt)
                nc.vector.tensor_tensor(st[:], cf[:], gt[:], ALU.subtract)
                nc.vector.tensor_scalar(st[:], st[:], 0.0, float(G - WIN), ALU.max, ALU.min)
                # s16[m, b] = 10*st0[b] + st1[b], broadcast to 16 partitions
                s16 = pmps.tile([16, 128], F32, tag="ps_z")
                nc.tensor.matmul(s16[:], comb2_sb[:], st[:], start=True, stop=True)
                # per-partition diagonal extract: base16[p, q] = s16[p, q*16+p]
                nc.vector.tensor_tensor(s16m[:], s16[:], mask_sb[:], ALU.mult)
                nc.vector.reduce_sum(base16[:],
                                     s16m[:].rearrange("p (q i) -> p q i", i=16),
                                     axis=mybir.AxisListType.X)
                nc.vector.tensor_tensor(base16[:], base16[:], iota_sb[:], ALU.add)
                for j in range(WIN):
                    nc.vector.tensor_scalar_add(idxf[:, j * 8:(j + 1) * 8], base16[:],
                                                float(G * j))
                nc.vector.tensor_copy(idx16[:], idxf[:])
                for r in range(8):
                    nc.sync.dma_start(idxrep[16 * r:16 * (r + 1), :], idx16[:])

                # ---- w2h = (h @ W2)^T  [uo*128+m, b] ----
                for uo in range(KU):
                    psw = pmps.tile([128, 128], F32, tag="ps_w2h")
                    for ui in range(KU):
                        nc.tensor.matmul(
                            psw[:],
                            w2_sb[:, ui * U + uo * 128: ui * U + (uo + 1) * 128],
                            hT16[:, ui * 128:(ui + 1) * 128],
                            start=(ui == 0), stop=(ui == KU - 1))
                    nc.vector.tensor_copy(w2h_sb[:, uo * 128:(uo + 1) * 128], psw[:])

            # ---- gather the 3x3 windows (3 gathers: one grid-row each) ----
            feat_gap = AP(feat.ap().tensor, 0, [[C, NROWS - 2], [1, WIN * C]])
            for j in range(WIN):
                g = nc.gpsimd.dma_gather(
                    local_nat[:, j * WIN * C:(j + 1) * WIN * C]
                        .rearrange("p (o e) -> p o e", o=1),
                    feat_gap,
                    idxrep[:, j * 8:(j + 1) * 8],
                    BS, BS, WIN * C,
                    elem_step=C,
                )
                add_dep_helper(g.ins, lib_inst.ins, reason="load_library before gather")

            # cast to bf16 + one multi-tile xbar transpose per grid row:
            # localT[c0, (l*KC+cc)*128 + b] = local_bf[b, l*C + cc*128 + c0]
            localT3 = localT[:].rearrange("p (t b) -> p t b", b=128)
            for j in range(WIN):
                nc.vector.tensor_copy(local_bf[:, j * WIN * C:(j + 1) * WIN * C],
                                      local_nat[:, j * WIN * C:(j + 1) * WIN * C])
                nc.sync.dma_start(
                    localT3[:, j * WIN * KC:(j + 1) * WIN * KC, :],
                    local_bf[:, j * WIN * C:(j + 1) * WIN * C],
                    transpose=True)

            # ---- scoreT = tanh(W1^T localT + w2h + b12)  [u, l*128+b] ----
            localT4 = localT[:].rearrange("p (l k b) -> p l k b", k=KC, b=128)
            with tc.tile_pool(name="sps", bufs=2, space="PSUM") as sps:
                for uo in range(KU):
                    pss = [sps.tile([128, 384], F32, tag=f"ps_s{j}",
                                    name=f"pss{uo}_{j}")
                           for j in range(WIN)]
                    for k in range(KC):
                        for j in range(WIN):
                            nc.tensor.matmul(
                                pss[j][:],
                                w1_sb[:, k * U + uo * 128:k * U + (uo + 1) * 128],
                                localT4[:, 3 * j:3 * j + 3, k, :],
                                start=(k == 0), stop=(k == KC - 1))
                    for j in range(WIN):
                        nc.vector.tensor_tensor(
                            stmp[:, j * 384:(j + 1) * 384]
                                .rearrange("p (l b) -> p l b", b=128),
                            pss[j][:].rearrange("p (l b) -> p l b", b=128),
                            w2h_sb[:, uo * 128:(uo + 1) * 128].unsqueeze(1)
                                .broadcast_to([128, WIN, 128]),
                            ALU.add)
                    nc.scalar.activation(scoreT[:, uo * RPW:(uo + 1) * RPW], stmp[:],
                                         AF.Tanh, bias=b12_sb[:, uo:uo + 1])

            # ---- logits -> softmax -> attn ----
            with tc.tile_pool(name="lgps", bufs=1, space="PSUM") as lgps:
                lg = lgps.tile([128, NL], F32, tag="ps_lg")
                for l in range(NL):
                    for uo in range(KU):
                        nc.tensor.matmul(
                            lg[:, l:l + 1],
                            scoreT[:, uo * RPW + l * 128:uo * RPW + (l + 1) * 128],
                            v1_sb[:, uo:uo + 1],
                            start=(uo == 0), stop=(uo == KU - 1))
                nc.vector.tensor_reduce(negmax[:], lg[:], axis=mybir.AxisListType.X,
                                        op=ALU.max, negate=True)
                nc.scalar.activation(e_sb[:], lg[:], AF.Exp, bias=negmax[:])
            nc.vector.reduce_sum(esum[:], e_sb[:], axis=mybir.AxisListType.X)
            nc.vector.reciprocal(rsum[:], esum[:])
            nc.vector.scalar_tensor_tensor(attn_f[:], e_sb[:], rsum[:], gauss_sb[:],
                                           ALU.mult, ALU.mult)
            nc.scalar.dma_start(attn[:], attn_f[:])

            # ---- ctx^T via diag matmuls, then tanh -> tct16 ----
            for l in range(NL):
                nc.vector.tensor_scalar_mul(diag[:, l * 128:(l + 1) * 128], eye_sb[:],
                                            attn_f[:, l:l + 1])
            with tc.tile_pool(name="cps", bufs=4, space="PSUM") as cps, \
                 tc.tile_pool(name="ops", bufs=1, space="PSUM") as ops:
                for cc in range(KC):
                    pc = cps.tile([128, 128], F32, tag="ps_c")
                    for l in range(NL):
                        nc.tensor.matmul(pc[:],
                                         local_bf[:, l * C + cc * 128:l * C + (cc + 1) * 128],
                                         diag[:, l * 128:(l + 1) * 128],
                                         start=(l == 0), stop=(l == NL - 1))
                    nc.scalar.activation(tct16[:, cc * 128:(cc + 1) * 128], pc[:], AF.Tanh)

                # ---- out = tanh([ctx, h]) @ W3 + W3_b ----
                po = ops.tile([128, U], F32, tag="ps_o")
                for kk in range(KT):
                    lhsT = (tct16[:, kk * 128:(kk + 1) * 128] if kk < KC
                            else th16[:, (kk - KC) * 128:(kk - KC + 1) * 128])
                    nc.tensor.matmul(po[:], lhsT, w3_sb[:, kk * U:(kk + 1) * U],
                                     start=(kk == 0), stop=(kk == KT - 1))
                nc.vector.tensor_tensor(out_sb[:], po[:], w3b_sb[:], ALU.add)
            nc.scalar.dma_start(out[:], out_sb[:])

    nc.compile()
    return nc


_NC_CACHE = None


def _get_nc():
    global _NC_CACHE
    if _NC_CACHE is None:
        _NC_CACHE = _build_nc()
    return _NC_CACHE


def make_host_inputs(features, hidden, W1_w, W1_b, W2_w, W2_b, V1_w, V1_b,
                     W3_w, W3_b, Wa, Wb):
    """Build the 8 per-core input maps."""
    bf = ml_dtypes.bfloat16
    f = np.float32

    jj, kk = np.meshgrid(np.arange(WIN), np.arange(WIN), indexing="ij")
    d2 = (jj - WIN / 2.0) ** 2 + (kk - WIN / 2.0) ** 2
    gauss_row = np.exp(-d2 / (0.5 * D * D)).reshape(NL).astype(f)

    p = np.arange(16)
    q = np.arange(8)
    iota = (L * (q[None, :] * 16 + p[:, None])).astype(f)

    shared = {
        "w1": np.ascontiguousarray(W1_w, f).astype(bf),
        "w2": np.ascontiguousarray(W2_w, f).astype(bf),
        "w3": np.ascontiguousarray(W3_w, f).astype(bf),
        "wa": np.ascontiguousarray(Wa, f),
        "wb": np.ascontiguousarray(Wb, f),
        "v1": np.ascontiguousarray(V1_w, f).astype(bf),
        "b12": (np.asarray(W1_b, f) + np.asarray(W2_b, f)).reshape(U, 1),
        "w3b": np.broadcast_to(np.asarray(W3_b, f), (128, U)).copy(),
        "iota": iota,
        "eye": np.eye(128, dtype=f).astype(bf),
        "gauss": np.broadcast_to(gauss_row, (128, NL)).copy(),
        "id32": np.eye(128, dtype=f),
        "comb2": np.stack([np.full(16, float(G), f), np.ones(16, f)]),
        "mask16": (np.arange(128)[None, :] % 16 == np.arange(16)[:, None])
                  .astype(f),
    }
    features = np.asarray(features, f)
    hidden = np.asarray(hidden, f)
    in_maps = []
    for c in range(NC_CORES):
        sl = slice(c * BS, (c + 1) * BS)
        m = dict(shared)
        m["feat"] = np.ascontiguousarray(features[sl]).reshape(NROWS, C)
        m["hid"] = np.ascontiguousarray(hidden[sl])
        in_maps.append(m)
    return in_maps


def kernel(features, hidden, W1_w, W1_b, W2_w, W2_b, V1_w, V1_b,
           W3_w, W3_b, Wa, Wb, _run_kwargs=None):
    nc = _get_nc()
    in_maps = make_host_inputs(features, hidden, W1_w, W1_b, W2_w, W2_b,
                               V1_w, V1_b, W3_w, W3_b, Wa, Wb)
    res = run_bass_kernel_spmd(nc, in_maps, core_ids=list(range(NC_CORES)),
                               **(_run_kwargs or {}))
    out = np.concatenate([r["out"] for r in res.results], axis=0)
    attn = np.concatenate([r["attn"] for r in res.results], axis=0)
    kernel.last_results = res
    return out, attn.reshape(B, NL, 1)


# revision 20
# speedup vs baseline: 1.8111x; 1.0085x over previous
"""Trainium2 Bass kernel for local-window Bahdanau attention.

Problem (hardcoded shapes): B=1024, L=100 (10x10 grid), C=1024, U=512,
window 3x3 (D=1).  Reference computes:
    p_t   = sigmoid(tanh(h @ Wa) @ Wb) * 8 + 1          (B,1,2)
    st    = int32(p_t - 1) clamped to [0, 7]            (B,2)
    local = grid[b, st0:st0+3, st1:st1+3, :]            (B,9,C)
    score = tanh(local @ W1 + W1_b + h @ W2 + W2_b)     (B,9,U)
    attn  = softmax(score @ V1 + V1_b, axis=1) * gauss  (B,9,1)
    ctx   = sum(attn * local, axis=1)                   (B,C)
    out   = tanh(concat([ctx, h])) @ W3 + W3_b          (B,U)
returns (out, attn).

Strategy: pure data-parallel over 8 NeuronCores (128 examples each).
The 3x3 window rows are fetched with dma_gather (device-side int16
indices computed from hidden), so only 9/100 of `features` is read.
Matmul-heavy parts run in bf16 (fp32 PSUM accumulation); the index
computation path (p_t) is kept entirely in fp32 to match the
reference's truncation.
"""

import sys

if "/opt/trn_rl_repo" not in sys.path:
    sys.path.insert(0, "/opt/trn_rl_repo")

import numpy as np
import ml_dtypes

import concourse.bass as bass
import concourse.bacc as bacc
import concourse.mybir as mybir
from concourse.ap import AP
from concourse.tile import TileContext, add_dep_helper
from concourse.bass_utils import run_bass_kernel_spmd
from concourse.library_config import mlp

F32 = mybir.dt.float32
BF16 = mybir.dt.bfloat16
I16 = mybir.dt.int16
I32 = mybir.dt.int32
AF = mybir.ActivationFunctionType
ALU = mybir.AluOpType

B, L, C, U = 1024, 100, 1024, 512
G, WIN, D = 10, 3, 1
NL = WIN * WIN            # 9 window positions
NC_CORES = 8
BS = B // NC_CORES        # 128 examples per core
NROWS = BS * L            # 12800 feature rows per core
KC = C // 128             # 8 contraction chunks over C
KU = U // 128             # 4 chunks over U
KT = (C + U) // 128       # 12 chunks over C+U
RPW = NL * BS             # 1152 score rows per core

# packed bf16 weights layout (column offsets in the [128, .] tile)
W_W1, W_W2, W_W3 = 0, KC * U, (KC + KU) * U
W_EYE = (KC + KU + KT) * U
W_V1 = W_EYE + 128
W_COLS = W_V1 + KU
# packed f32 consts layout
F_WA, F_W3B = 0, KU * 100
F_GAUSS = F_W3B + U
F_IOTA = F_GAUSS + NL
F_MASK = F_IOTA + 8
F_ID = F_MASK + 128
F_B12 = F_ID + 128
F_COMB = F_B12 + KU
F_COLS = F_COMB + 128


def _build_nc():
    nc = bacc.Bacc("TRN2", target_bir_lowering=False)

    feat = nc.dram_tensor("feat", [NROWS, C], F32, kind="ExternalInput")
    hid = nc.dram_tensor("hid", [BS, U], F32, kind="ExternalInput")
    wcat = nc.dram_tensor("wcat", [128, W_COLS], BF16, kind="ExternalInput")
    fcat = nc.dram_tensor("fcat", [128, F_COLS], F32, kind="ExternalInput")
    wbd = nc.dram_tensor("wbd", [100, 2], F32, kind="ExternalInput")

    out = nc.dram_tensor("out", [BS, U], F32, kind="ExternalOutput")
    attn = nc.dram_tensor("attn", [BS, NL], F32, kind="ExternalOutput")

    with TileContext(nc) as tc:
        lib_inst = nc.gpsimd.load_library(mlp)

        with tc.tile_pool(name="sb", bufs=1) as sb:
            # ---- persistent SBUF tiles ----
            h_nat = sb.tile([BS, U], F32, tag="h_nat")
            hT32 = sb.tile([128, U], F32, tag="hT32")
            hT16 = sb.tile([128, U], BF16, tag="hT16")
            th16 = sb.tile([128, U], BF16, tag="th16")
            w_all = sb.tile([128, W_COLS], BF16, tag="w_all")
            f_all = sb.tile([128, F_COLS], F32, tag="f_all")
            wb_sb = sb.tile([100, 2], F32, tag="wb_sb")

            w1_sb = w_all[:, W_W1:W_W1 + KC * U]
            w2_sb = w_all[:, W_W2:W_W2 + KU * U]
            w3_sb = w_all[:, W_W3:W_W3 + KT * U]
            eye_sb = w_all[:, W_EYE:W_EYE + 128]
            v1_sb = w_all[:, W_V1:W_V1 + KU]
            wa_sb = f_all[:, F_WA:F_WA + KU * 100]
            w3b_sb = f_all[:, F_W3B:F_W3B + U]
            gauss_sb = f_all[:, F_GAUSS:F_GAUSS + NL]
            iota_sb = f_all[:, F_IOTA:F_IOTA + 8]
            mask_sb = f_all[:, F_MASK:F_MASK + 128]
            id_sb = f_all[:, F_ID:F_ID + 128]
            b12_sb = f_all[:, F_B12:F_B12 + KU]
            comb2_sb = f_all[0:2, F_COMB:F_COMB + 128]

            local_nat = sb.tile([128, NL * C], F32, tag="local_nat")
            local_bf = sb.tile([128, NL * C], BF16, tag="local_bf")
            localT = sb.tile([128, KC * RPW], BF16, tag="localT")
            w2h_sb = sb.tile([128, KU * 128], F32, tag="w2h_sb")
            scoreT = sb.tile([128, KU * RPW], BF16, tag="scoreT")
            tct16 = sb.tile([128, KC * 128], BF16, tag="tct16")
            diag = sb.tile([128, NL * 128], BF16, tag="diag")
            out_sb = sb.tile([BS, U], F32, tag="out_sb")

            t1 = sb.tile([128, 128], F32, tag="t1")
            t2 = sb.tile([2, 128], F32, tag="t2")
            pm1 = sb.tile([2, 128], F32, tag="pm1")
            ci = sb.tile([2, 128], I32, tag="ci")
            cf = sb.tile([2, 128], F32, tag="cf")
            gt = sb.tile([2, 128], F32, tag="gt")
            st = sb.tile([2, 128], F32, tag="st")
            s16m = sb.tile([128, 128], F32, tag="s16m")
            base16 = sb.tile([128, 8], F32, tag="base16")
            idxf = sb.tile([128, 24], F32, tag="idxf")
            idx16 = sb.tile([128, 24], I16, tag="idx16")
            stmp = sb.tile([128, RPW], F32, tag="stmp")
            negmax = sb.tile([128, 1], F32, tag="negmax")
            esum = sb.tile([128, 1], F32, tag="esum")
            rsum = sb.tile([128, 1], F32, tag="rsum")
            e_sb = sb.tile([128, NL], F32, tag="e_sb")
            attn_f = sb.tile([128, NL], F32, tag="attn_f")

            # ---- input DMAs ----
            nc.sync.dma_start(h_nat[:], hid[:])
            nc.sync.dma_start(f_all[:], fcat[:])
            nc.sync.dma_start(wb_sb[:], wbd[:])
            nc.scalar.dma_start(w_all[:], wcat[:])

            # ---- hT (PE transpose, fp32) ----
            with tc.tile_pool(name="pmps", bufs=2, space="PSUM") as pmps:
                for k in range(KU):
                    ps = pmps.tile([128, 128], F32, tag="ps_tr", name=f"ptr{k}")
                    nc.tensor.transpose(ps[:], h_nat[:, k * 128:(k + 1) * 128], id_sb)
                    nc.vector.tensor_copy(hT32[:, k * 128:(k + 1) * 128], ps[:])
                nc.vector.tensor_copy(hT16[:], hT32[:])

                # ---- p_t chain (fp32 end to end) ----
                z1 = pmps.tile([128, 128], F32, tag="ps_z")
                for k in range(KU):
                    nc.tensor.matmul(z1[0:100, :], wa_sb[:, k * 100:(k + 1) * 100],
                                     hT32[:, k * 128:(k + 1) * 128],
                                     start=(k == 0), stop=(k == KU - 1))
                nc.scalar.activation(t1[0:100, :], z1[0:100, :], AF.Tanh)
                z2 = pmps.tile([128, 128], F32, tag="ps_z")
                nc.tensor.matmul(z2[0:2, :], wb_sb[:], t1[0:100, :], start=True, stop=True)
                # p_t - 1 = 8*sigmoid(z) = 4*tanh(z/2) + 4
                nc.scalar.activation(t2[:], z2[0:2, :], AF.Tanh, scale=0.5)
                nc.vector.tensor_scalar(pm1[:], t2[:], 4.0, 4.0, ALU.mult, ALU.add)
                # floor (rounding-mode agnostic): c=int(x); c -= (c > x)
                nc.vector.tensor_copy(ci[:], pm1[:])
                nc.vector.tensor_copy(cf[:], ci[:])
                nc.vector.tensor_tensor(gt[:], cf[:], pm1[:], ALU.is_gt)
                nc.vector.tensor_tensor(st[:], cf[:], gt[:], ALU.subtract)
                nc.vector.tensor_scalar(st[:], st[:], 0.0, float(G - WIN), ALU.max, ALU.min)
                # s16[m, b] = 10*st0[b] + st1[b] on all 128 partitions
                s16 = pmps.tile([128, 128], F32, tag="ps_z")
                nc.tensor.matmul(s16[:], comb2_sb, st[:], start=True, stop=True)
                # diagonal extract: base16[p, q] = s16[p, q*16 + p%16]
                nc.vector.tensor_tensor(s16m[:], s16[:], mask_sb, ALU.mult)
                nc.vector.reduce_sum(base16[:],
                                     s16m[:].rearrange("p (q i) -> p q i", i=16),
                                     axis=mybir.AxisListType.X)
                nc.vector.tensor_tensor(base16[:], base16[:], iota_sb, ALU.add)
                for j in range(WIN):
                    nc.vector.tensor_scalar_add(idxf[:, j * 8:(j + 1) * 8], base16[:],
                                                float(G * j))
                nc.vector.tensor_copy(idx16[:], idxf[:])

                # ---- w2h = (h @ W2)^T  [uo*128+m, b], one psum bank ----
                psw = pmps.tile([128, 512], F32, tag="ps_w2h")
                for uo in range(KU):
                    for ui in range(KU):
                        nc.tensor.matmul(
                            psw[:, uo * 128:(uo + 1) * 128],
                            w2_sb[:, ui * U + uo * 128: ui * U + (uo + 1) * 128],
                            hT16[:, ui * 128:(ui + 1) * 128],
                            start=(ui == 0), stop=(ui == KU - 1))
                nc.vector.tensor_copy(w2h_sb[:], psw[:])
                # tanh(h)^T for the W3 matmul tail
                nc.scalar.activation(th16[:], hT32[:], AF.Tanh)

            # ---- gather the 3x3 windows (3 gathers: one grid-row each) ----
            feat_gap = AP(feat.ap().tensor, 0, [[C, NROWS - 2], [1, WIN * C]])
            for j in range(WIN):
                g = nc.gpsimd.dma_gather(
                    local_nat[:, j * WIN * C:(j + 1) * WIN * C]
                        .rearrange("p (o e) -> p o e", o=1),
                    feat_gap,
                    idx16[:, j * 8:(j + 1) * 8],
                    BS, BS, WIN * C,
                    elem_step=C,
                )
                add_dep_helper(g.ins, lib_inst.ins, reason="load_library before gather")

            # cast to bf16 + one multi-tile xbar transpose per grid row:
            # localT[c0, (l*KC+cc)*128 + b] = local_bf[b, l*C + cc*128 + c0]
            localT3 = localT[:].rearrange("p (t b) -> p t b", b=128)
            for j in range(WIN):
                nc.vector.tensor_copy(local_bf[:, j * WIN * C:(j + 1) * WIN * C],
                                      local_nat[:, j * WIN * C:(j + 1) * WIN * C])
                nc.sync.dma_start(
                    localT3[:, j * WIN * KC:(j + 1) * WIN * KC, :],
                    local_bf[:, j * WIN * C:(j + 1) * WIN * C],
                    transpose=True)

            # ---- scoreT = tanh(W1^T localT + w2h + b12)  [u, l*128+b] ----
            localT4 = localT[:].rearrange("p (l k b) -> p l k b", k=KC, b=128)
            with tc.tile_pool(name="sps", bufs=2, space="PSUM") as sps:
                for uo in range(KU):
                    pss = [sps.tile([128, 384], F32, tag=f"ps_s{j}",
                                    name=f"pss{uo}_{j}")
                           for j in range(WIN)]
                    for k in range(KC):
                        for j in range(WIN):
                            nc.tensor.matmul(
                                pss[j][:],
                                w1_sb[:, k * U + uo * 128:k * U + (uo + 1) * 128],
                                localT4[:, 3 * j:3 * j + 3, k, :],
                                start=(k == 0), stop=(k == KC - 1))
                    for j in range(WIN):
                        nc.vector.tensor_tensor(
                            stmp[:, j * 384:(j + 1) * 384]
                                .rearrange("p (l b) -> p l b", b=128),
                            pss[j][:].rearrange("p (l b) -> p l b", b=128),
                            w2h_sb[:, uo * 128:(uo + 1) * 128].unsqueeze(1)
                                .broadcast_to([128, WIN, 128]),
                            ALU.add)
                    nc.scalar.activation(scoreT[:, uo * RPW:(uo + 1) * RPW], stmp[:],
                                         AF.Tanh, bias=b12_sb[:, uo:uo + 1])

            # ---- logits -> softmax -> attn ----
            with tc.tile_pool(name="lgps", bufs=1, space="PSUM") as lgps:
                lg = lgps.tile([128, NL], F32, tag="ps_lg")
                for l in range(NL):
                    for uo in range(KU):
                        nc.tensor.matmul(
                            lg[:, l:l + 1],
                            scoreT[:, uo * RPW + l * 128:uo * RPW + (l + 1) * 128],
                            v1_sb[:, uo:uo + 1],
                            start=(uo == 0), stop=(uo == KU - 1))
                nc.vector.tensor_reduce(negmax[:], lg[:], axis=mybir.AxisListType.X,
                                        op=ALU.max, negate=True)
                nc.scalar.activation(e_sb[:], lg[:], AF.Exp, bias=negmax[:])
            nc.vector.reduce_sum(esum[:], e_sb[:], axis=mybir.AxisListType.X)
            nc.vector.reciprocal(rsum[:], esum[:])
            nc.vector.scalar_tensor_tensor(attn_f[:], e_sb[:], rsum[:], gauss_sb,
                                           ALU.mult, ALU.mult)
            nc.scalar.dma_start(attn[:], attn_f[:])

            # ---- ctx^T via diag matmuls, then tanh -> tct16 ----
            for l in range(NL):
                nc.vector.tensor_scalar_mul(diag[:, l * 128:(l + 1) * 128], eye_sb,
                                            attn_f[:, l:l + 1])
            with tc.tile_pool(name="cps", bufs=4, space="PSUM") as cps, \
                 tc.tile_pool(name="ops", bufs=1, space="PSUM") as ops:
                for cc in range(KC):
                    pc = cps.tile([128, 128], F32, tag="ps_c", name=f"pc{cc}")
                    for l in range(NL):
                        nc.tensor.matmul(pc[:],
                                         local_bf[:, l * C + cc * 128:l * C + (cc + 1) * 128],
                                         diag[:, l * 128:(l + 1) * 128],
                                         start=(l == 0), stop=(l == NL - 1))
                    nc.scalar.activation(tct16[:, cc * 128:(cc + 1) * 128], pc[:], AF.Tanh)

                # ---- out = tanh([ctx, h]) @ W3 + W3_b ----
                po = ops.tile([128, U], F32, tag="ps_o")
                for kk in range(KT):
                    lhsT = (tct16[:, kk * 128:(kk + 1) * 128] if kk < KC
                            else th16[:, (kk - KC) * 128:(kk - KC + 1) * 128])
                    nc.tensor.matmul(po[:], lhsT, w3_sb[:, kk * U:(kk + 1) * U],
                                     start=(kk == 0), stop=(kk == KT - 1))
                nc.vector.tensor_tensor(out_sb[:], po[:], w3b_sb, ALU.add)
            nc.scalar.dma_start(out[:], out_sb[:])

    nc.compile()
    return nc


_NC_CACHE = None


def _get_nc():
    global _NC_CACHE
    if _NC_CACHE is None:
        _NC_CACHE = _build_nc()
    return _NC_CACHE


def _chunked(w, k):
    """[k*128, n] -> [128, k*n] with chunk-major columns."""
    n = w.shape[1]
    return np.ascontiguousarray(
        w.reshape(k, 128, n).transpose(1, 0, 2).reshape(128, k * n))


def make_host_inputs(features, hidden, W1_w, W1_b, W2_w, W2_b, V1_w, V1_b,
                     W3_w, W3_b, Wa, Wb):
    """Build the 8 per-core input maps."""
    bf = ml_dtypes.bfloat16
    f = np.float32

    jj, kk = np.meshgrid(np.arange(WIN), np.arange(WIN), indexing="ij")
    d2 = (jj - WIN / 2.0) ** 2 + (kk - WIN / 2.0) ** 2
    gauss_row = np.exp(-d2 / (0.5 * D * D)).reshape(NL).astype(f)

    p = np.arange(128)
    q = np.arange(8)

    wcat = np.zeros((128, W_COLS), bf)
    wcat[:, W_W1:W_W2] = _chunked(np.asarray(W1_w, f), KC).astype(bf)
    wcat[:, W_W2:W_W3] = _chunked(np.asarray(W2_w, f), KU).astype(bf)
    wcat[:, W_W3:W_EYE] = _chunked(np.asarray(W3_w, f), KT).astype(bf)
    wcat[:, W_EYE:W_V1] = np.eye(128, dtype=f).astype(bf)
    wcat[:, W_V1:W_COLS] = _chunked(np.asarray(V1_w, f), KU).astype(bf)

    fcat = np.zeros((128, F_COLS), f)
    fcat[:, F_WA:F_W3B] = _chunked(np.asarray(Wa, f), KU)
    fcat[:, F_W3B:F_GAUSS] = np.broadcast_to(np.asarray(W3_b, f), (128, U))
    fcat[:, F_GAUSS:F_IOTA] = np.broadcast_to(gauss_row, (128, NL))
    fcat[:, F_IOTA:F_MASK] = L * (q[None, :] * 16 + (p[:, None] % 16))
    fcat[:, F_MASK:F_ID] = (np.arange(128)[None, :] % 16 == p[:, None] % 16)
    fcat[:, F_ID:F_B12] = np.eye(128, dtype=f)
    fcat[:, F_B12:F_COMB] = _chunked(
        (np.asarray(W1_b, f) + np.asarray(W2_b, f)).reshape(U, 1), KU)
    fcat[0, F_COMB:F_COLS] = float(G)
    fcat[1, F_COMB:F_COLS] = 1.0

    shared = {
        "wcat": wcat,
        "fcat": fcat,
        "wbd": np.ascontiguousarray(Wb, f),
    }
    features = np.asarray(features, f)
    hidden = np.asarray(hidden, f)
    in_maps = []
    for c in range(NC_CORES):
        sl = slice(c * BS, (c + 1) * BS)
        m = dict(shared)
        m["feat"] = np.ascontiguousarray(features[sl]).reshape(NROWS, C)
        m["hid"] = np.ascontiguousarray(hidden[sl])
        in_maps.append(m)
    return in_maps


def kernel(features, hidden, W1_w, W1_b, W2_w, W2_b, V1_w, V1_b,
           W3_w, W3_b, Wa, Wb, _run_kwargs=None):
    nc = _get_nc()
    in_maps = make_host_inputs(features, hidden, W1_w, W1_b, W2_w, W2_b,
                               V1_w, V1_b, W3_w, W3_b, Wa, Wb)
    res = run_bass_kernel_spmd(nc, in_maps, core_ids=list(range(NC_CORES)),
                               **(_run_kwargs or {}))
    out = np.concatenate([r["out"] for r in res.results], axis=0)
    attn = np.concatenate([r["attn"] for r in res.results], axis=0)
    kernel.last_results = res
    return out, attn.reshape(B, NL, 1)


# revision 22
# speedup vs baseline: 2.0544x; 1.1344x over previous
"""Trainium2 Bass kernel for local-window Bahdanau attention.

Problem (hardcoded shapes): B=1024, L=100 (10x10 grid), C=1024, U=512,
window 3x3 (D=1).  Reference computes:
    p_t   = sigmoid(tanh(h @ Wa) @ Wb) * 8 + 1          (B,1,2)
    st    = int32(p_t - 1) clamped to [0, 7]            (B,2)
    local = grid[b, st0:st0+3, st1:st1+3, :]            (B,9,C)
    score = tanh(local @ W1 + W1_b + h @ W2 + W2_b)     (B,9,U)
    attn  = softmax(score @ V1 + V1_b, axis=1) * gauss  (B,9,1)
    ctx   = sum(attn * local, axis=1)                   (B,C)
    out   = tanh(concat([ctx, h])) @ W3 + W3_b          (B,U)
returns (out, attn).

Strategy: pure data-parallel over 8 NeuronCores (128 examples each).
The 3x3 window rows are fetched with dma_gather (device-side int16
indices computed from hidden), so only 9/100 of `features` is read.
Matmul-heavy parts run in bf16 (fp32 PSUM accumulation); the index
computation path (p_t) is kept entirely in fp32 to match the
reference's truncation.
"""

import sys

if "/opt/trn_rl_repo" not in sys.path:
    sys.path.insert(0, "/opt/trn_rl_repo")

import numpy as np
import ml_dtypes

import concourse.bass as bass
import concourse.bacc as bacc
import concourse.mybir as mybir
from concourse.ap import AP
from concourse.tile import TileContext, add_dep_helper
from concourse.bass_utils import run_bass_kernel_spmd
from concourse.library_config import mlp

F32 = mybir.dt.float32
BF16 = mybir.dt.bfloat16
I16 = mybir.dt.int16
I32 = mybir.dt.int32
AF = mybir.ActivationFunctionType
ALU = mybir.AluOpType

B, L, C, U = 1024, 100, 1024, 512
G, WIN, D = 10, 3, 1
NL = WIN * WIN            # 9 window positions
NC_CORES = 8
BS = B // NC_CORES        # 128 examples per core
NROWS = BS * L            # 12800 feature rows per core
KC = C // 128             # 8 contraction chunks over C
KU = U // 128             # 4 chunks over U
KT = (C + U) // 128       # 12 chunks over C+U
RPW = NL * BS             # 1152 score rows per core

# packed bf16 weights layout (column offsets in the [128, .] tile)
W_W1, W_W2, W_W3 = 0, KC * U, (KC + KU) * U
W_EYE = (KC + KU + KT) * U
W_V1 = W_EYE + 128
W_COLS = W_V1 + KU
W2_OFF = 0          # w2cat: [w2]
# critical f32 consts (needed for the p_t/index chain)
F_ID, F_WA = 0, 128
F_MASK = F_WA + KU * 100
F_COMB = F_MASK + 128
FC_COLS = F_COMB + 128
# late f32 consts
F_W3B = 0
F_GAUSS = F_W3B + U
F_IOTA = F_GAUSS + NL
F_B12 = F_IOTA + 8
FR_COLS = F_B12 + KU


def _build_nc():
    nc = bacc.Bacc("TRN2", target_bir_lowering=False)

    feat = nc.dram_tensor("feat", [NROWS, C], F32, kind="ExternalInput")
    hid = nc.dram_tensor("hid", [BS, U], F32, kind="ExternalInput")
    fcrit = nc.dram_tensor("fcrit", [128, FC_COLS], F32, kind="ExternalInput")
    w2cat = nc.dram_tensor("w2cat", [128, KU * U], BF16, kind="ExternalInput")
    w1cat = nc.dram_tensor("w1cat", [128, KC * U], BF16, kind="ExternalInput")
    frest = nc.dram_tensor("frest", [128, FR_COLS], F32, kind="ExternalInput")
    w3cat = nc.dram_tensor("w3cat", [128, KT * U + 128 + KU], BF16,
                           kind="ExternalInput")
    wbd = nc.dram_tensor("wbd", [100, 2], F32, kind="ExternalInput")

    out = nc.dram_tensor("out", [BS, U], F32, kind="ExternalOutput")
    attn = nc.dram_tensor("attn", [BS, NL], F32, kind="ExternalOutput")

    with TileContext(nc) as tc:
        lib_inst = nc.gpsimd.load_library(mlp)

        with tc.tile_pool(name="sb", bufs=1) as sb:
            # ---- persistent SBUF tiles ----
            h_nat = sb.tile([BS, U], F32, tag="h_nat")
            hT32 = sb.tile([128, U], F32, tag="hT32")
            hT16 = sb.tile([128, U], BF16, tag="hT16")
            th16 = sb.tile([128, U], BF16, tag="th16")
            w_all = sb.tile([128, W_COLS], BF16, tag="w_all")
            fc_all = sb.tile([128, FC_COLS], F32, tag="fc_all")
            fr_all = sb.tile([128, FR_COLS], F32, tag="fr_all")
            wb_sb = sb.tile([100, 2], F32, tag="wb_sb")

            w1_sb = w_all[:, W_W1:W_W1 + KC * U]
            w2_sb = w_all[:, W_W2:W_W2 + KU * U]
            w3_sb = w_all[:, W_W3:W_W3 + KT * U]
            eye_sb = w_all[:, W_EYE:W_EYE + 128]
            v1_sb = w_all[:, W_V1:W_V1 + KU]
            id_sb = fc_all[:, F_ID:F_ID + 128]
            wa_sb = fc_all[:, F_WA:F_WA + KU * 100]
            mask_sb = fc_all[:, F_MASK:F_MASK + 128]
            comb2_sb = fc_all[0:2, F_COMB:F_COMB + 128]
            w3b_sb = fr_all[:, F_W3B:F_W3B + U]
            gauss_sb = fr_all[:, F_GAUSS:F_GAUSS + NL]
            iota_sb = fr_all[:, F_IOTA:F_IOTA + 8]
            b12_sb = fr_all[:, F_B12:F_B12 + KU]

            local_nat = sb.tile([128, NL * C], F32, tag="local_nat")
            local_bf = sb.tile([128, NL * C], BF16, tag="local_bf")
            localT = sb.tile([128, KC * RPW], BF16, tag="localT")
            w2h_sb = sb.tile([128, KU * 128], F32, tag="w2h_sb")
            scoreT = sb.tile([128, KU * RPW], BF16, tag="scoreT")
            tct16 = sb.tile([128, KC * 128], BF16, tag="tct16")
            diag = sb.tile([128, NL * 128], BF16, tag="diag")
            out_sb = sb.tile([BS, U], F32, tag="out_sb")

            t1 = sb.tile([128, 128], F32, tag="t1")
            t2 = sb.tile([2, 128], F32, tag="t2")
            pm1 = sb.tile([2, 128], F32, tag="pm1")
            ci = sb.tile([2, 128], I32, tag="ci")
            cf = sb.tile([2, 128], F32, tag="cf")
            gt = sb.tile([2, 128], F32, tag="gt")
            st = sb.tile([2, 128], F32, tag="st")
            s16m = sb.tile([128, 128], F32, tag="s16m")
            base16 = sb.tile([128, 8], F32, tag="base16")
            idxf = sb.tile([128, 24], F32, tag="idxf")
            idx16 = sb.tile([128, 24], I16, tag="idx16")
            stmp = sb.tile([128, RPW], F32, tag="stmp")
            negmax = sb.tile([128, 1], F32, tag="negmax")
            esum = sb.tile([128, 1], F32, tag="esum")
            rsum = sb.tile([128, 1], F32, tag="rsum")
            e_sb = sb.tile([128, NL], F32, tag="e_sb")
            attn_f = sb.tile([128, NL], F32, tag="attn_f")

            # ---- input DMAs: one FIFO ring (Sync), priority order ----
            nc.sync.dma_start(h_nat[:], hid[:])
            nc.sync.dma_start(fc_all[:], fcrit[:])
            nc.sync.dma_start(wb_sb[:], wbd[:])
            nc.sync.dma_start(w_all[:, W_W2:W_W2 + KU * U], w2cat[:])
            nc.sync.dma_start(w_all[:, W_W1:W_W1 + KC * U], w1cat[:])
            nc.sync.dma_start(fr_all[:], frest[:])
            nc.sync.dma_start(w_all[:, W_W3:], w3cat[:])

            # ---- hT (PE transpose, fp32) ----
            with tc.tile_pool(name="pmps", bufs=2, space="PSUM") as pmps:
                for k in range(KU):
                    ps = pmps.tile([128, 128], F32, tag="ps_tr", name=f"ptr{k}")
                    nc.tensor.transpose(ps[:], h_nat[:, k * 128:(k + 1) * 128], id_sb)
                    nc.vector.tensor_copy(hT32[:, k * 128:(k + 1) * 128], ps[:])
                nc.vector.tensor_copy(hT16[:], hT32[:])

                # ---- p_t chain (fp32 end to end) ----
                z1 = pmps.tile([128, 128], F32, tag="ps_z")
                for k in range(KU):
                    nc.tensor.matmul(z1[0:100, :], wa_sb[:, k * 100:(k + 1) * 100],
                                     hT32[:, k * 128:(k + 1) * 128],
                                     start=(k == 0), stop=(k == KU - 1))
                nc.scalar.activation(t1[0:100, :], z1[0:100, :], AF.Tanh)
                z2 = pmps.tile([128, 128], F32, tag="ps_z")
                nc.tensor.matmul(z2[0:2, :], wb_sb[:], t1[0:100, :], start=True, stop=True)
                # p_t - 1 = 8*sigmoid(z) = 4*tanh(z/2) + 4
                nc.scalar.activation(t2[:], z2[0:2, :], AF.Tanh, scale=0.5)
                nc.vector.tensor_scalar(pm1[:], t2[:], 4.0, 4.0, ALU.mult, ALU.add)
                # floor (rounding-mode agnostic): c=int(x); c -= (c > x)
                nc.vector.tensor_copy(ci[:], pm1[:])
                nc.vector.tensor_copy(cf[:], ci[:])
                nc.vector.tensor_tensor(gt[:], cf[:], pm1[:], ALU.is_gt)
                nc.vector.tensor_tensor(st[:], cf[:], gt[:], ALU.subtract)
                nc.vector.tensor_scalar(st[:], st[:], 0.0, float(G - WIN), ALU.max, ALU.min)
                # s16[m, b] = 10*st0[b] + st1[b] on all 128 partitions
                s16 = pmps.tile([128, 128], F32, tag="ps_z")
                nc.tensor.matmul(s16[:], comb2_sb, st[:], start=True, stop=True)
                # diagonal extract: base16[p, q] = s16[p, q*16 + p%16]
                nc.vector.tensor_tensor(s16m[:], s16[:], mask_sb, ALU.mult)
                nc.vector.reduce_sum(base16[:],
                                     s16m[:].rearrange("p (q i) -> p q i", i=16),
                                     axis=mybir.AxisListType.X)
                nc.vector.tensor_tensor(base16[:], base16[:], iota_sb, ALU.add)
                for j in range(WIN):
                    nc.vector.tensor_scalar_add(idxf[:, j * 8:(j + 1) * 8], base16[:],
                                                float(G * j))
                nc.vector.tensor_copy(idx16[:], idxf[:])

                # ---- w2h = (h @ W2)^T  [uo*128+m, b], one psum bank ----
                psw = pmps.tile([128, 512], F32, tag="ps_w2h")
                for uo in range(KU):
                    for ui in range(KU):
                        nc.tensor.matmul(
                            psw[:, uo * 128:(uo + 1) * 128],
                            w2_sb[:, ui * U + uo * 128: ui * U + (uo + 1) * 128],
                            hT16[:, ui * 128:(ui + 1) * 128],
                            start=(ui == 0), stop=(ui == KU - 1))
                nc.vector.tensor_copy(w2h_sb[:], psw[:])
                # tanh(h)^T for the W3 matmul tail
                nc.scalar.activation(th16[:], hT32[:], AF.Tanh)

            # ---- gather the 3x3 windows (3 gathers: one grid-row each) ----
            feat_gap = AP(feat.ap().tensor, 0, [[C, NROWS - 2], [1, WIN * C]])
            for j in range(WIN):
                g = nc.gpsimd.dma_gather(
                    local_nat[:, j * WIN * C:(j + 1) * WIN * C]
                        .rearrange("p (o e) -> p o e", o=1),
                    feat_gap,
                    idx16[:, j * 8:(j + 1) * 8],
                    BS, BS, WIN * C,
                    elem_step=C,
                )
                add_dep_helper(g.ins, lib_inst.ins, reason="load_library before gather")

            # cast to bf16 + one multi-tile xbar transpose per grid row:
            # localT[c0, (l*KC+cc)*128 + b] = local_bf[b, l*C + cc*128 + c0]
            localT3 = localT[:].rearrange("p (t b) -> p t b", b=128)
            for j in range(WIN):
                nc.vector.tensor_copy(local_bf[:, j * WIN * C:(j + 1) * WIN * C],
                                      local_nat[:, j * WIN * C:(j + 1) * WIN * C])
            for j in range(WIN):
                nc.sync.dma_start(
                    localT3[:, j * WIN * KC:(j + 1) * WIN * KC, :],
                    local_bf[:, j * WIN * C:(j + 1) * WIN * C],
                    transpose=True)

            # ---- scoreT = tanh(W1^T localT + w2h + b12)  [u, l*128+b] ----
            localT4 = localT[:].rearrange("p (l k b) -> p l k b", k=KC, b=128)
            with tc.tile_pool(name="sps", bufs=2, space="PSUM") as sps:
                for uo in range(KU):
                    pss = [sps.tile([128, 384], F32, tag=f"ps_s{j}",
                                    name=f"pss{uo}_{j}")
                           for j in range(WIN)]
                    for k in range(KC):
                        for j in range(WIN):
                            nc.tensor.matmul(
                                pss[j][:],
                                w1_sb[:, k * U + uo * 128:k * U + (uo + 1) * 128],
                                localT4[:, 3 * j:3 * j + 3, k, :],
                                start=(k == 0), stop=(k == KC - 1))
                    for j in range(WIN):
                        nc.vector.tensor_tensor(
                            stmp[:, j * 384:(j + 1) * 384]
                                .rearrange("p (l b) -> p l b", b=128),
                            pss[j][:].rearrange("p (l b) -> p l b", b=128),
                            w2h_sb[:, uo * 128:(uo + 1) * 128].unsqueeze(1)
                                .broadcast_to([128, WIN, 128]),
                            ALU.add)
                    nc.scalar.activation(scoreT[:, uo * RPW:(uo + 1) * RPW], stmp[:],
                                         AF.Tanh, bias=b12_sb[:, uo:uo + 1])

            # ---- logits -> softmax -> attn ----
            with tc.tile_pool(name="lgps", bufs=1, space="PSUM") as lgps:
                lg = lgps.tile([128, NL], F32, tag="ps_lg")
                for l in range(NL):
                    for uo in range(KU):
                        nc.tensor.matmul(
                            lg[:, l:l + 1],
                            scoreT[:, uo * RPW + l * 128:uo * RPW + (l + 1) * 128],
                            v1_sb[:, uo:uo + 1],
                            start=(uo == 0), stop=(uo == KU - 1))
                nc.vector.tensor_reduce(negmax[:], lg[:], axis=mybir.AxisListType.X,
                                        op=ALU.max, negate=True)
                nc.scalar.activation(e_sb[:], lg[:], AF.Exp, bias=negmax[:])
            nc.vector.reduce_sum(esum[:], e_sb[:], axis=mybir.AxisListType.X)
            nc.vector.reciprocal(rsum[:], esum[:])
            nc.vector.scalar_tensor_tensor(attn_f[:], e_sb[:], rsum[:], gauss_sb,
                                           ALU.mult, ALU.mult)
            nc.sync.dma_start(attn[:], attn_f[:])

            # ---- ctx^T via diag matmuls, then tanh -> tct16 ----
            for l in range(NL):
                nc.vector.tensor_scalar_mul(diag[:, l * 128:(l + 1) * 128], eye_sb,
                                            attn_f[:, l:l + 1])
            with tc.tile_pool(name="cps", bufs=4, space="PSUM") as cps, \
                 tc.tile_pool(name="ops", bufs=1, space="PSUM") as ops:
                for cc in range(KC):
                    pc = cps.tile([128, 128], F32, tag="ps_c", name=f"pc{cc}")
                    for l in range(NL):
                        nc.tensor.matmul(pc[:],
                                         local_bf[:, l * C + cc * 128:l * C + (cc + 1) * 128],
                                         diag[:, l * 128:(l + 1) * 128],
                                         start=(l == 0), stop=(l == NL - 1))
                    nc.scalar.activation(tct16[:, cc * 128:(cc + 1) * 128], pc[:], AF.Tanh)

                # ---- out = tanh([ctx, h]) @ W3 + W3_b ----
                po = ops.tile([128, U], F32, tag="ps_o")
                for kk in range(KT):
                    lhsT = (tct16[:, kk * 128:(kk + 1) * 128] if kk < KC
                            else th16[:, (kk - KC) * 128:(kk - KC + 1) * 128])
                    nc.tensor.matmul(po[:], lhsT, w3_sb[:, kk * U:(kk + 1) * U],
                                     start=(kk == 0), stop=(kk == KT - 1))
                nc.vector.tensor_tensor(out_sb[:], po[:], w3b_sb, ALU.add)
            nc.sync.dma_start(out[:], out_sb[:])

    nc.compile()
    return nc


_NC_CACHE = None


def _get_nc():
    global _NC_CACHE
    if _NC_CACHE is None:
        _NC_CACHE = _build_nc()
    return _NC_CACHE


def _chunked(w, k):
    """[k*128, n] -> [128, k*n] with chunk-major columns."""
    n = w.shape[1]
    return np.ascontiguousarray(
        w.reshape(k, 128, n).transpose(1, 0, 2).reshape(128, k * n))


def make_host_inputs(features, hidden, W1_w, W1_b, W2_w, W2_b, V1_w, V1_b,
                     W3_w, W3_b, Wa, Wb):
    """Build the 8 per-core input maps."""
    bf = ml_dtypes.bfloat16
    f = np.float32

    jj, kk = np.meshgrid(np.arange(WIN), np.arange(WIN), indexing="ij")
    d2 = (jj - WIN / 2.0) ** 2 + (kk - WIN / 2.0) ** 2
    gauss_row = np.exp(-d2 / (0.5 * D * D)).reshape(NL).astype(f)

    p = np.arange(128)
    q = np.arange(8)

    w3cat = np.zeros((128, KT * U + 128 + KU), bf)
    w3cat[:, 0:KT * U] = _chunked(np.asarray(W3_w, f), KT).astype(bf)
    w3cat[:, KT * U:KT * U + 128] = np.eye(128, dtype=f).astype(bf)
    w3cat[:, KT * U + 128:] = _chunked(np.asarray(V1_w, f), KU).astype(bf)

    fcrit = np.zeros((128, FC_COLS), f)
    fcrit[:, F_ID:F_ID + 128] = np.eye(128, dtype=f)
    fcrit[:, F_WA:F_WA + KU * 100] = _chunked(np.asarray(Wa, f), KU)
    fcrit[:, F_MASK:F_MASK + 128] = (
        np.arange(128)[None, :] % 16 == p[:, None] % 16)
    fcrit[0, F_COMB:F_COMB + 128] = float(G)
    fcrit[1, F_COMB:F_COMB + 128] = 1.0

    frest = np.zeros((128, FR_COLS), f)
    frest[:, F_W3B:F_GAUSS] = np.broadcast_to(np.asarray(W3_b, f), (128, U))
    frest[:, F_GAUSS:F_IOTA] = np.broadcast_to(gauss_row, (128, NL))
    frest[:, F_IOTA:F_B12] = L * (q[None, :] * 16 + (p[:, None] % 16))
    frest[:, F_B12:F_B12 + KU] = _chunked(
        (np.asarray(W1_b, f) + np.asarray(W2_b, f)).reshape(U, 1), KU)

    shared = {
        "fcrit": fcrit,
        "frest": frest,
        "w1cat": _chunked(np.asarray(W1_w, f), KC).astype(bf),
        "w2cat": _chunked(np.asarray(W2_w, f), KU).astype(bf),
        "w3cat": w3cat,
        "wbd": np.ascontiguousarray(Wb, f),
    }
    features = np.asarray(features, f)
    hidden = np.asarray(hidden, f)
    in_maps = []
    for c in range(NC_CORES):
        sl = slice(c * BS, (c + 1) * BS)
        m = dict(shared)
        m["feat"] = np.ascontiguousarray(features[sl]).reshape(NROWS, C)
        m["hid"] = np.ascontiguousarray(hidden[sl])
        in_maps.append(m)
    return in_maps


def kernel(features, hidden, W1_w, W1_b, W2_w, W2_b, V1_w, V1_b,
           W3_w, W3_b, Wa, Wb, _run_kwargs=None):
    nc = _get_nc()
    in_maps = make_host_inputs(features, hidden, W1_w, W1_b, W2_w, W2_b,
                               V1_w, V1_b, W3_w, W3_b, Wa, Wb)
    res = run_bass_kernel_spmd(nc, in_maps, core_ids=list(range(NC_CORES)),
                               **(_run_kwargs or {}))
    out = np.concatenate([r["out"] for r in res.results], axis=0)
    attn = np.concatenate([r["attn"] for r in res.results], axis=0)
    kernel.last_results = res
    return out, attn.reshape(B, NL, 1)


# revision 23
# speedup vs baseline: 2.0774x; 1.0112x over previous
"""Trainium2 Bass kernel for local-window Bahdanau attention.

Problem (hardcoded shapes): B=1024, L=100 (10x10 grid), C=1024, U=512,
window 3x3 (D=1).  Reference computes:
    p_t   = sigmoid(tanh(h @ Wa) @ Wb) * 8 + 1          (B,1,2)
    st    = int32(p_t - 1) clamped to [0, 7]            (B,2)
    local = grid[b, st0:st0+3, st1:st1+3, :]            (B,9,C)
    score = tanh(local @ W1 + W1_b + h @ W2 + W2_b)     (B,9,U)
    attn  = softmax(score @ V1 + V1_b, axis=1) * gauss  (B,9,1)
    ctx   = sum(attn * local, axis=1)                   (B,C)
    out   = tanh(concat([ctx, h])) @ W3 + W3_b          (B,U)
returns (out, attn).

Strategy: pure data-parallel over 8 NeuronCores (128 examples each).
The 3x3 window rows are fetched with dma_gather (device-side int16
indices computed from hidden), so only 9/100 of `features` is read.
Matmul-heavy parts run in bf16 (fp32 PSUM accumulation); the index
computation path (p_t) is kept entirely in fp32 to match the
reference's truncation.
"""

import sys

if "/opt/trn_rl_repo" not in sys.path:
    sys.path.insert(0, "/opt/trn_rl_repo")

import numpy as np
import ml_dtypes

import concourse.bass as bass
import concourse.bacc as bacc
import concourse.mybir as mybir
from concourse.ap import AP
from concourse.tile import TileContext, add_dep_helper
from concourse.bass_utils import run_bass_kernel_spmd
from concourse.library_config import mlp

F32 = mybir.dt.float32
BF16 = mybir.dt.bfloat16
I16 = mybir.dt.int16
I32 = mybir.dt.int32
AF = mybir.ActivationFunctionType
ALU = mybir.AluOpType

B, L, C, U = 1024, 100, 1024, 512
G, WIN, D = 10, 3, 1
NL = WIN * WIN            # 9 window positions
NC_CORES = 8
BS = B // NC_CORES        # 128 examples per core
NROWS = BS * L            # 12800 feature rows per core
KC = C // 128             # 8 contraction chunks over C
KU = U // 128             # 4 chunks over U
KT = (C + U) // 128       # 12 chunks over C+U
RPW = NL * BS             # 1152 score rows per core

# packed bf16 weights layout (column offsets in the [128, .] tile)
W_W1, W_W2, W_W3 = 0, KC * U, (KC + KU) * U
W_EYE = (KC + KU + KT) * U
W_V1 = W_EYE + 128
W_COLS = W_V1 + KU
W2_OFF = 0          # w2cat: [w2]
# critical f32 consts (needed for the p_t/index chain)
F_ID, F_WA = 0, 128
F_MASK = F_WA + KU * 100
F_COMB = F_MASK + 128
FC_COLS = F_COMB + 128
# late f32 consts
F_W3B = 0
F_GAUSS = F_W3B + U
F_IOTA = F_GAUSS + NL
F_B12 = F_IOTA + 8
FR_COLS = F_B12 + KU


def _build_nc():
    nc = bacc.Bacc("TRN2", target_bir_lowering=False)

    feat = nc.dram_tensor("feat", [NROWS, C], F32, kind="ExternalInput")
    hid = nc.dram_tensor("hid", [BS, U], F32, kind="ExternalInput")
    fcrit = nc.dram_tensor("fcrit", [128, FC_COLS], F32, kind="ExternalInput")
    w2cat = nc.dram_tensor("w2cat", [128, KU * U], BF16, kind="ExternalInput")
    w1cat = nc.dram_tensor("w1cat", [128, KC * U], BF16, kind="ExternalInput")
    frest = nc.dram_tensor("frest", [128, FR_COLS], F32, kind="ExternalInput")
    w3cat = nc.dram_tensor("w3cat", [128, KT * U + 128 + KU], BF16,
                           kind="ExternalInput")
    wbd = nc.dram_tensor("wbd", [100, 2], F32, kind="ExternalInput")

    out = nc.dram_tensor("out", [BS, U], F32, kind="ExternalOutput")
    attn = nc.dram_tensor("attn", [BS, NL], F32, kind="ExternalOutput")

    with TileContext(nc) as tc:
        lib_inst = nc.gpsimd.load_library(mlp)

        with tc.tile_pool(name="sb", bufs=1) as sb:
            # ---- persistent SBUF tiles ----
            h_nat = sb.tile([BS, U], F32, tag="h_nat")
            hT32 = sb.tile([128, U], F32, tag="hT32")
            hT16 = sb.tile([128, U], BF16, tag="hT16")
            th16 = sb.tile([128, U], BF16, tag="th16")
            w_all = sb.tile([128, W_COLS], BF16, tag="w_all")
            fc_all = sb.tile([128, FC_COLS], F32, tag="fc_all")
            fr_all = sb.tile([128, FR_COLS], F32, tag="fr_all")
            wb_sb = sb.tile([100, 2], F32, tag="wb_sb")

            w1_sb = w_all[:, W_W1:W_W1 + KC * U]
            w2_sb = w_all[:, W_W2:W_W2 + KU * U]
            w3_sb = w_all[:, W_W3:W_W3 + KT * U]
            eye_sb = w_all[:, W_EYE:W_EYE + 128]
            v1_sb = w_all[:, W_V1:W_V1 + KU]
            id_sb = fc_all[:, F_ID:F_ID + 128]
            wa_sb = fc_all[:, F_WA:F_WA + KU * 100]
            mask_sb = fc_all[:, F_MASK:F_MASK + 128]
            comb2_sb = fc_all[0:2, F_COMB:F_COMB + 128]
            w3b_sb = fr_all[:, F_W3B:F_W3B + U]
            gauss_sb = fr_all[:, F_GAUSS:F_GAUSS + NL]
            iota_sb = fr_all[:, F_IOTA:F_IOTA + 8]
            b12_sb = fr_all[:, F_B12:F_B12 + KU]

            local_nat = sb.tile([128, NL * C], F32, tag="local_nat")
            local_bf = sb.tile([128, NL * C], BF16, tag="local_bf")
            localT = sb.tile([128, KC * RPW], BF16, tag="localT")
            w2h_sb = sb.tile([128, KU * 128], F32, tag="w2h_sb")
            scoreT = sb.tile([128, KU * RPW], BF16, tag="scoreT")
            tct16 = sb.tile([128, KC * 128], BF16, tag="tct16")
            diag = sb.tile([128, NL * 128], BF16, tag="diag")
            out_sb = sb.tile([BS, U], F32, tag="out_sb")

            t1 = sb.tile([128, 128], F32, tag="t1")
            t2 = sb.tile([2, 128], F32, tag="t2")
            pm1 = sb.tile([2, 128], F32, tag="pm1")
            ci = sb.tile([2, 128], I32, tag="ci")
            cf = sb.tile([2, 128], F32, tag="cf")
            gt = sb.tile([2, 128], F32, tag="gt")
            st = sb.tile([2, 128], F32, tag="st")
            s16m = sb.tile([128, 128], F32, tag="s16m")
            base16 = sb.tile([128, 8], F32, tag="base16")
            idxf = sb.tile([128, 24], F32, tag="idxf")
            idx16 = sb.tile([128, 24], I16, tag="idx16")
            stmp = sb.tile([128, RPW], F32, tag="stmp")
            negmax = sb.tile([128, 1], F32, tag="negmax")
            esum = sb.tile([128, 1], F32, tag="esum")
            rsum = sb.tile([128, 1], F32, tag="rsum")
            e_sb = sb.tile([128, NL], F32, tag="e_sb")
            attn_f = sb.tile([128, NL], F32, tag="attn_f")

            # ---- input DMAs: one FIFO ring (Sync), priority order ----
            nc.sync.dma_start(h_nat[:], hid[:])
            nc.sync.dma_start(fc_all[:], fcrit[:])
            nc.sync.dma_start(wb_sb[:], wbd[:])
            nc.sync.dma_start(w_all[:, W_W2:W_W2 + KU * U], w2cat[:])
            nc.sync.dma_start(w_all[:, W_W1:W_W1 + KC * U], w1cat[:])
            nc.sync.dma_start(fr_all[:], frest[:])
            nc.sync.dma_start(w_all[:, W_W3:], w3cat[:])

            # ---- hT (PE transpose, fp32) ----
            with tc.tile_pool(name="pmps", bufs=2, space="PSUM") as pmps:
                for k in range(KU):
                    ps = pmps.tile([128, 128], F32, tag="ps_tr", name=f"ptr{k}")
                    nc.tensor.transpose(ps[:], h_nat[:, k * 128:(k + 1) * 128], id_sb)
                    nc.vector.tensor_copy(hT32[:, k * 128:(k + 1) * 128], ps[:])
                # ---- p_t chain (fp32 end to end) ----
                z1 = pmps.tile([128, 128], F32, tag="ps_z")
                for k in range(KU):
                    nc.tensor.matmul(z1[0:100, :], wa_sb[:, k * 100:(k + 1) * 100],
                                     hT32[:, k * 128:(k + 1) * 128],
                                     start=(k == 0), stop=(k == KU - 1))
                nc.scalar.activation(t1[0:100, :], z1[0:100, :], AF.Tanh)
                z2 = pmps.tile([128, 128], F32, tag="ps_z")
                nc.tensor.matmul(z2[0:2, :], wb_sb[:], t1[0:100, :], start=True, stop=True)
                # p_t - 1 = 8*sigmoid(z) = 4*tanh(z/2) + 4
                nc.scalar.activation(t2[:], z2[0:2, :], AF.Tanh, scale=0.5)
                nc.vector.tensor_scalar(pm1[:], t2[:], 4.0, 4.0, ALU.mult, ALU.add)
                # floor (rounding-mode agnostic): c=int(x); c -= (c > x)
                nc.vector.tensor_copy(ci[:], pm1[:])
                nc.vector.tensor_copy(cf[:], ci[:])
                nc.vector.tensor_tensor(gt[:], cf[:], pm1[:], ALU.is_gt)
                nc.vector.tensor_tensor(st[:], cf[:], gt[:], ALU.subtract)
                # (clamp omitted: p_t-1 in (0,8) strictly, so floor in [0,7])
                # s16[m, b] = 10*st0[b] + st1[b] on all 128 partitions
                s16 = pmps.tile([128, 128], F32, tag="ps_z")
                nc.tensor.matmul(s16[:], comb2_sb, st[:], start=True, stop=True)
                # diagonal extract: base16[p, q] = s16[p, q*16 + p%16]
                nc.vector.tensor_tensor(s16m[:], s16[:], mask_sb, ALU.mult)
                nc.vector.reduce_sum(base16[:],
                                     s16m[:].rearrange("p (q i) -> p q i", i=16),
                                     axis=mybir.AxisListType.X)
                for j in range(WIN):
                    nc.vector.scalar_tensor_tensor(
                        idx16[:, j * 8:(j + 1) * 8], base16[:], float(G * j),
                        iota_sb, ALU.add, ALU.add)

            # ---- gather the 3x3 windows (3 gathers: one grid-row each) ----
            feat_gap = AP(feat.ap().tensor, 0, [[C, NROWS - 2], [1, WIN * C]])
            gathers = []
            for j in range(WIN):
                g = nc.gpsimd.dma_gather(
                    local_nat[:, j * WIN * C:(j + 1) * WIN * C]
                        .rearrange("p (o e) -> p o e", o=1),
                    feat_gap,
                    idx16[:, j * 8:(j + 1) * 8],
                    BS, BS, WIN * C,
                    elem_step=C,
                )
                add_dep_helper(g.ins, lib_inst.ins, reason="load_library before gather")
                gathers.append(g)

            with tc.tile_pool(name="pm2", bufs=2, space="PSUM") as pm2:
                # ---- w2h = (h @ W2)^T  [uo*128+m, b], one psum bank ----
                nc.vector.tensor_copy(hT16[:], hT32[:])
                psw = pm2.tile([128, 512], F32, tag="ps_w2h")
                for uo in range(KU):
                    for ui in range(KU):
                        nc.tensor.matmul(
                            psw[:, uo * 128:(uo + 1) * 128],
                            w2_sb[:, ui * U + uo * 128: ui * U + (uo + 1) * 128],
                            hT16[:, ui * 128:(ui + 1) * 128],
                            start=(ui == 0), stop=(ui == KU - 1))
                nc.vector.tensor_copy(w2h_sb[:], psw[:])
                # tanh(h)^T for the W3 matmul tail
                nc.scalar.activation(th16[:], hT32[:], AF.Tanh)

                # PE warmup during the gather/transpose window (keeps HAM at
                # full clock); results are never read.
                warm = pm2.tile([128, 512], F32, tag="ps_warm")
                for i in range(48):
                    nc.tensor.matmul(warm[:], w1_sb[:, 0:128],
                                     w1_sb[:, (i % 7) * 512:(i % 7) * 512 + 512],
                                     start=True, stop=True)

            # cast to bf16 + one multi-tile xbar transpose per grid row:
            # localT[c0, (l*KC+cc)*128 + b] = local_bf[b, l*C + cc*128 + c0]
            localT3 = localT[:].rearrange("p (t b) -> p t b", b=128)
            for j in range(WIN):
                nc.vector.tensor_copy(local_bf[:, j * WIN * C:(j + 1) * WIN * C],
                                      local_nat[:, j * WIN * C:(j + 1) * WIN * C])
            for j in range(WIN):
                tr = nc.sync.dma_start(
                    localT3[:, j * WIN * KC:(j + 1) * WIN * KC, :],
                    local_bf[:, j * WIN * C:(j + 1) * WIN * C],
                    transpose=True)
                for g in gathers:
                    add_dep_helper(tr.ins, g.ins,
                                   reason="xbar transpose after all gathers")

            # ---- scoreT = tanh(W1^T localT + w2h + b12)  [u, l*128+b] ----
            localT4 = localT[:].rearrange("p (l k b) -> p l k b", k=KC, b=128)
            REG = ((0, 4), (4, 8), (8, 9))   # l-ranges: N = 512, 512, 128
            with tc.tile_pool(name="sps", bufs=2, space="PSUM") as sps:
                for uo in range(KU):
                    pss = sps.tile([128, RPW], F32, tag="ps_s", name=f"pss{uo}")
                    for k in range(KC):
                        for (l0, l1) in REG:
                            nc.tensor.matmul(
                                pss[:, l0 * 128:l1 * 128],
                                w1_sb[:, k * U + uo * 128:k * U + (uo + 1) * 128],
                                localT4[:, l0:l1, k, :],
                                start=(k == 0), stop=(k == KC - 1))
                    nc.vector.tensor_tensor(
                        stmp[:].rearrange("p (l b) -> p l b", b=128),
                        pss[:].rearrange("p (l b) -> p l b", b=128),
                        w2h_sb[:, uo * 128:(uo + 1) * 128].unsqueeze(1)
                            .broadcast_to([128, NL, 128]),
                        ALU.add)
                    nc.scalar.activation(scoreT[:, uo * RPW:(uo + 1) * RPW], stmp[:],
                                         AF.Tanh, bias=b12_sb[:, uo:uo + 1])

            # ---- logits -> softmax -> attn ----
            with tc.tile_pool(name="lgps", bufs=1, space="PSUM") as lgps:
                lg = lgps.tile([128, NL], F32, tag="ps_lg")
                for l in range(NL):
                    for uo in range(KU):
                        nc.tensor.matmul(
                            lg[:, l:l + 1],
                            scoreT[:, uo * RPW + l * 128:uo * RPW + (l + 1) * 128],
                            v1_sb[:, uo:uo + 1],
                            start=(uo == 0), stop=(uo == KU - 1))
                nc.vector.tensor_reduce(negmax[:], lg[:], axis=mybir.AxisListType.X,
                                        op=ALU.max, negate=True)
                nc.scalar.activation(e_sb[:], lg[:], AF.Exp, bias=negmax[:])
            nc.vector.reduce_sum(esum[:], e_sb[:], axis=mybir.AxisListType.X)
            nc.vector.reciprocal(rsum[:], esum[:])
            nc.vector.scalar_tensor_tensor(attn_f[:], e_sb[:], rsum[:], gauss_sb,
                                           ALU.mult, ALU.mult)
            nc.sync.dma_start(attn[:], attn_f[:])

            # ---- ctx^T via diag matmuls, then tanh -> tct16 ----
            for l in range(NL):
                nc.vector.tensor_scalar_mul(diag[:, l * 128:(l + 1) * 128], eye_sb,
                                            attn_f[:, l:l + 1])
            with tc.tile_pool(name="cps", bufs=4, space="PSUM") as cps, \
                 tc.tile_pool(name="ops", bufs=1, space="PSUM") as ops:
                for cc in range(KC):
                    pc = cps.tile([128, 128], F32, tag="ps_c", name=f"pc{cc}")
                    for l in range(NL):
                        nc.tensor.matmul(pc[:],
                                         local_bf[:, l * C + cc * 128:l * C + (cc + 1) * 128],
                                         diag[:, l * 128:(l + 1) * 128],
                                         start=(l == 0), stop=(l == NL - 1))
                    nc.scalar.activation(tct16[:, cc * 128:(cc + 1) * 128], pc[:], AF.Tanh)

                # ---- out = tanh([ctx, h]) @ W3 + W3_b ----
                po = ops.tile([128, U], F32, tag="ps_o")
                for kk in range(KT):
                    lhsT = (tct16[:, kk * 128:(kk + 1) * 128] if kk < KC
                            else th16[:, (kk - KC) * 128:(kk - KC + 1) * 128])
                    nc.tensor.matmul(po[:], lhsT, w3_sb[:, kk * U:(kk + 1) * U],
                                     start=(kk == 0), stop=(kk == KT - 1))
                nc.vector.tensor_tensor(out_sb[:], po[:], w3b_sb, ALU.add)
            nc.sync.dma_start(out[:], out_sb[:])

    nc.compile()
    return nc


_NC_CACHE = None


def _get_nc():
    global _NC_CACHE
    if _NC_CACHE is None:
        _NC_CACHE = _build_nc()
    return _NC_CACHE


def _chunked(w, k):
    """[k*128, n] -> [128, k*n] with chunk-major columns."""
    n = w.shape[1]
    return np.ascontiguousarray(
        w.reshape(k, 128, n).transpose(1, 0, 2).reshape(128, k * n))


def make_host_inputs(features, hidden, W1_w, W1_b, W2_w, W2_b, V1_w, V1_b,
                     W3_w, W3_b, Wa, Wb):
    """Build the 8 per-core input maps."""
    bf = ml_dtypes.bfloat16
    f = np.float32

    jj, kk = np.meshgrid(np.arange(WIN), np.arange(WIN), indexing="ij")
    d2 = (jj - WIN / 2.0) ** 2 + (kk - WIN / 2.0) ** 2
    gauss_row = np.exp(-d2 / (0.5 * D * D)).reshape(NL).astype(f)

    p = np.arange(128)
    q = np.arange(8)

    w3cat = np.zeros((128, KT * U + 128 + KU), bf)
    w3cat[:, 0:KT * U] = _chunked(np.asarray(W3_w, f), KT).astype(bf)
    w3cat[:, KT * U:KT * U + 128] = np.eye(128, dtype=f).astype(bf)
    w3cat[:, KT * U + 128:] = _chunked(np.asarray(V1_w, f), KU).astype(bf)

    fcrit = np.zeros((128, FC_COLS), f)
    fcrit[:, F_ID:F_ID + 128] = np.eye(128, dtype=f)
    fcrit[:, F_WA:F_WA + KU * 100] = _chunked(np.asarray(Wa, f), KU)
    fcrit[:, F_MASK:F_MASK + 128] = (
        np.arange(128)[None, :] % 16 == p[:, None] % 16)
    fcrit[0, F_COMB:F_COMB + 128] = float(G)
    fcrit[1, F_COMB:F_COMB + 128] = 1.0

    frest = np.zeros((128, FR_COLS), f)
    frest[:, F_W3B:F_GAUSS] = np.broadcast_to(np.asarray(W3_b, f), (128, U))
    frest[:, F_GAUSS:F_IOTA] = np.broadcast_to(gauss_row, (128, NL))
    frest[:, F_IOTA:F_B12] = L * (q[None, :] * 16 + (p[:, None] % 16))
    frest[:, F_B12:F_B12 + KU] = _chunked(
        (np.asarray(W1_b, f) + np.asarray(W2_b, f)).reshape(U, 1), KU)

    shared = {
        "fcrit": fcrit,
        "frest": frest,
        "w1cat": _chunked(np.asarray(W1_w, f), KC).astype(bf),
        "w2cat": _chunked(np.asarray(W2_w, f), KU).astype(bf),
        "w3cat": w3cat,
        "wbd": np.ascontiguousarray(Wb, f),
    }
    features = np.asarray(features, f)
    hidden = np.asarray(hidden, f)
    in_maps = []
    for c in range(NC_CORES):
        sl = slice(c * BS, (c + 1) * BS)
        m = dict(shared)
        m["feat"] = np.ascontiguousarray(features[sl]).reshape(NROWS, C)
        m["hid"] = np.ascontiguousarray(hidden[sl])
        in_maps.append(m)
    return in_maps


def kernel(features, hidden, W1_w, W1_b, W2_w, W2_b, V1_w, V1_b,
           W3_w, W3_b, Wa, Wb, _run_kwargs=None):
    nc = _get_nc()
    in_maps = make_host_inputs(features, hidden, W1_w, W1_b, W2_w, W2_b,
                               V1_w, V1_b, W3_w, W3_b, Wa, Wb)
    res = run_bass_kernel_spmd(nc, in_maps, core_ids=list(range(NC_CORES)),
                               **(_run_kwargs or {}))
    out = np.concatenate([r["out"] for r in res.results], axis=0)
    attn = np.concatenate([r["attn"] for r in res.results], axis=0)
    kernel.last_results = res
    return out, attn.reshape(B, NL, 1)


# revision 24
# speedup vs baseline: 2.2849x; 1.0999x over previous
"""Trainium2 Bass kernel for local-window Bahdanau attention.

Problem (hardcoded shapes): B=1024, L=100 (10x10 grid), C=1024, U=512,
window 3x3 (D=1).  Reference computes:
    p_t   = sigmoid(tanh(h @ Wa) @ Wb) * 8 + 1          (B,1,2)
    st    = int32(p_t - 1) clamped to [0, 7]            (B,2)
    local = grid[b, st0:st0+3, st1:st1+3, :]            (B,9,C)
    score = tanh(local @ W1 + W1_b + h @ W2 + W2_b)     (B,9,U)
    attn  = softmax(score @ V1 + V1_b, axis=1) * gauss  (B,9,1)
    ctx   = sum(attn * local, axis=1)                   (B,C)
    out   = tanh(concat([ctx, h])) @ W3 + W3_b          (B,U)
returns (out, attn).

Strategy: pure data-parallel over 8 NeuronCores (128 examples each).
The 3x3 window rows are fetched with dma_gather (device-side int16
indices computed from hidden), so only 9/100 of `features` is read.
Matmul-heavy parts run in bf16 (fp32 PSUM accumulation); the index
computation path (p_t) is kept entirely in fp32 to match the
reference's truncation.
"""

import sys

if "/opt/trn_rl_repo" not in sys.path:
    sys.path.insert(0, "/opt/trn_rl_repo")

import numpy as np
import ml_dtypes

import concourse.bass as bass
import concourse.bacc as bacc
import concourse.mybir as mybir
from concourse.ap import AP
from concourse.tile import TileContext, add_dep_helper
from concourse.bass_utils import run_bass_kernel_spmd
from concourse.library_config import mlp

F32 = mybir.dt.float32
BF16 = mybir.dt.bfloat16
I16 = mybir.dt.int16
I32 = mybir.dt.int32
AF = mybir.ActivationFunctionType
ALU = mybir.AluOpType

B, L, C, U = 1024, 100, 1024, 512
G, WIN, D = 10, 3, 1
NL = WIN * WIN            # 9 window positions
NC_CORES = 8
BS = B // NC_CORES        # 128 examples per core
NROWS = BS * L            # 12800 feature rows per core
KC = C // 128             # 8 contraction chunks over C
KU = U // 128             # 4 chunks over U
KT = (C + U) // 128       # 12 chunks over C+U
RPW = NL * BS             # 1152 score rows per core

# packed bf16 weights layout (column offsets in the [128, .] tile)
W_W1, W_W2, W_W3 = 0, KC * U, (KC + KU) * U
W_EYE = (KC + KU + KT) * U
W_V1 = W_EYE + 128
W_COLS = W_V1 + KU
W2_OFF = 0          # w2cat: [w2]
# critical f32 consts (needed for the p_t/index chain)
F_ID, F_WA = 0, 128
F_MASK = F_WA + KU * 100
F_COMB = F_MASK + 128
FC_COLS = F_COMB + 128
# late f32 consts
F_W3B = 0
F_GAUSS = F_W3B + U
F_IOTA = F_GAUSS + NL
F_B12 = F_IOTA + 8
FR_COLS = F_B12 + KU


def _build_nc():
    nc = bacc.Bacc("TRN2", target_bir_lowering=False)

    feat = nc.dram_tensor("feat", [NROWS, C], F32, kind="ExternalInput")
    hid = nc.dram_tensor("hid", [BS, U], F32, kind="ExternalInput")
    fcrit = nc.dram_tensor("fcrit", [128, FC_COLS], F32, kind="ExternalInput")
    w2cat = nc.dram_tensor("w2cat", [128, KU * U], BF16, kind="ExternalInput")
    w1cat = nc.dram_tensor("w1cat", [128, KC * U], BF16, kind="ExternalInput")
    frest = nc.dram_tensor("frest", [128, FR_COLS], F32, kind="ExternalInput")
    w3cat = nc.dram_tensor("w3cat", [128, KT * U + 128 + KU], BF16,
                           kind="ExternalInput")
    wbd = nc.dram_tensor("wbd", [100, 2], F32, kind="ExternalInput")

    out = nc.dram_tensor("out", [BS, U], F32, kind="ExternalOutput")
    attn = nc.dram_tensor("attn", [BS, NL], F32, kind="ExternalOutput")

    with TileContext(nc) as tc:
        lib_inst = nc.gpsimd.load_library(mlp)

        with tc.tile_pool(name="sb", bufs=1) as sb:
            # ---- persistent SBUF tiles ----
            h_nat = sb.tile([BS, U], F32, tag="h_nat")
            hT32 = sb.tile([128, U], F32, tag="hT32")
            hT16 = sb.tile([128, U], BF16, tag="hT16")
            th16 = sb.tile([128, U], BF16, tag="th16")
            w_all = sb.tile([128, W_COLS], BF16, tag="w_all")
            fc_all = sb.tile([128, FC_COLS], F32, tag="fc_all")
            fr_all = sb.tile([128, FR_COLS], F32, tag="fr_all")
            wb_sb = sb.tile([100, 2], F32, tag="wb_sb")

            w1_sb = w_all[:, W_W1:W_W1 + KC * U]
            w2_sb = w_all[:, W_W2:W_W2 + KU * U]
            w3_sb = w_all[:, W_W3:W_W3 + KT * U]
            eye_sb = w_all[:, W_EYE:W_EYE + 128]
            v1_sb = w_all[:, W_V1:W_V1 + KU]
            id_sb = fc_all[:, F_ID:F_ID + 128]
            wa_sb = fc_all[:, F_WA:F_WA + KU * 100]
            mask_sb = fc_all[:, F_MASK:F_MASK + 128]
            comb2_sb = fc_all[0:2, F_COMB:F_COMB + 128]
            w3b_sb = fr_all[:, F_W3B:F_W3B + U]
            gauss_sb = fr_all[:, F_GAUSS:F_GAUSS + NL]
            iota_sb = fr_all[:, F_IOTA:F_IOTA + 8]
            b12_sb = fr_all[:, F_B12:F_B12 + KU]

            local_nat = sb.tile([128, NL * C], F32, tag="local_nat")
            local_bf = sb.tile([128, NL * C], BF16, tag="local_bf")
            localT = sb.tile([128, KC * RPW], BF16, tag="localT")
            w2h_sb = sb.tile([128, KU * 128], F32, tag="w2h_sb")
            scoreT = sb.tile([128, KU * RPW], BF16, tag="scoreT")
            tct16 = sb.tile([128, KC * 128], BF16, tag="tct16")
            diag = sb.tile([128, NL * 128], BF16, tag="diag")
            out_sb = sb.tile([BS, U], F32, tag="out_sb")

            t1 = sb.tile([128, 128], F32, tag="t1")
            t2 = sb.tile([2, 128], F32, tag="t2")
            pm1 = sb.tile([2, 128], F32, tag="pm1")
            ci = sb.tile([2, 128], I32, tag="ci")
            cf = sb.tile([2, 128], F32, tag="cf")
            gt = sb.tile([2, 128], F32, tag="gt")
            st = sb.tile([2, 128], F32, tag="st")
            s16m = sb.tile([128, 128], F32, tag="s16m")
            base16 = sb.tile([128, 8], F32, tag="base16")
            idxf = sb.tile([128, 24], F32, tag="idxf")
            idx16 = sb.tile([128, 24], I16, tag="idx16")
            stmp = sb.tile([128, RPW], F32, tag="stmp")
            negmax = sb.tile([128, 1], F32, tag="negmax")
            esum = sb.tile([128, 1], F32, tag="esum")
            rsum = sb.tile([128, 1], F32, tag="rsum")
            e_sb = sb.tile([128, NL], F32, tag="e_sb")
            attn_f = sb.tile([128, NL], F32, tag="attn_f")

            # ---- input DMAs: one FIFO ring (Sync), priority order ----
            nc.sync.dma_start(h_nat[:], hid[:])
            nc.sync.dma_start(fc_all[:], fcrit[:])
            nc.sync.dma_start(wb_sb[:], wbd[:])
            nc.sync.dma_start(w_all[:, W_W2:W_W2 + KU * U], w2cat[:])
            nc.sync.dma_start(w_all[:, W_W1:W_W1 + KC * U], w1cat[:])
            nc.sync.dma_start(fr_all[:], frest[:])
            nc.sync.dma_start(w_all[:, W_W3:], w3cat[:])

            # ---- hT (PE transpose, fp32) ----
            with tc.tile_pool(name="pmps", bufs=2, space="PSUM") as pmps:
                for k in range(KU):
                    ps = pmps.tile([128, 128], F32, tag="ps_tr", name=f"ptr{k}")
                    nc.tensor.transpose(ps[:], h_nat[:, k * 128:(k + 1) * 128], id_sb)
                    nc.vector.tensor_copy(hT32[:, k * 128:(k + 1) * 128], ps[:])
                # ---- p_t chain (fp32 end to end) ----
                z1 = pmps.tile([128, 128], F32, tag="ps_z")
                for k in range(KU):
                    nc.tensor.matmul(z1[0:100, :], wa_sb[:, k * 100:(k + 1) * 100],
                                     hT32[:, k * 128:(k + 1) * 128],
                                     start=(k == 0), stop=(k == KU - 1))
                nc.scalar.activation(t1[0:100, :], z1[0:100, :], AF.Tanh)
                z2 = pmps.tile([128, 128], F32, tag="ps_z")
                nc.tensor.matmul(z2[0:2, :], wb_sb[:], t1[0:100, :], start=True, stop=True)
                # p_t - 1 = 8*sigmoid(z) = 4*tanh(z/2) + 4
                nc.scalar.activation(t2[:], z2[0:2, :], AF.Tanh, scale=0.5)
                nc.vector.tensor_scalar(pm1[:], t2[:], 4.0, 4.0, ALU.mult, ALU.add)
                # floor (rounding-mode agnostic): c=int(x); c -= (c > x)
                nc.vector.tensor_copy(ci[:], pm1[:])
                nc.vector.tensor_copy(cf[:], ci[:])
                nc.vector.tensor_tensor(gt[:], cf[:], pm1[:], ALU.is_gt)
                nc.vector.tensor_tensor(st[:], cf[:], gt[:], ALU.subtract)
                # (clamp omitted: p_t-1 in (0,8) strictly, so floor in [0,7])
                # s16[m, b] = 10*st0[b] + st1[b] on all 128 partitions
                s16 = pmps.tile([128, 128], F32, tag="ps_z")
                nc.tensor.matmul(s16[:], comb2_sb, st[:], start=True, stop=True)
                # diagonal extract: base16[p, q] = s16[p, q*16 + p%16]
                nc.vector.tensor_tensor(s16m[:], s16[:], mask_sb, ALU.mult)
                nc.vector.reduce_sum(base16[:],
                                     s16m[:].rearrange("p (q i) -> p q i", i=16),
                                     axis=mybir.AxisListType.X)
                for j in range(WIN):
                    nc.vector.scalar_tensor_tensor(
                        idx16[:, j * 8:(j + 1) * 8], base16[:], float(G * j),
                        iota_sb, ALU.add, ALU.add)

            # ---- gather the 3x3 windows (3 gathers: one grid-row each) ----
            feat_gap = AP(feat.ap().tensor, 0, [[C, NROWS - 2], [1, WIN * C]])
            gathers = []
            for j in range(WIN):
                g = nc.gpsimd.dma_gather(
                    local_nat[:, j * WIN * C:(j + 1) * WIN * C]
                        .rearrange("p (o e) -> p o e", o=1),
                    feat_gap,
                    idx16[:, j * 8:(j + 1) * 8],
                    BS, BS, WIN * C,
                    elem_step=C,
                )
                add_dep_helper(g.ins, lib_inst.ins, reason="load_library before gather")
                gathers.append(g)

            with tc.tile_pool(name="pm2", bufs=2, space="PSUM") as pm2:
                # ---- w2h = (h @ W2)^T  [uo*128+m, b], one psum bank ----
                nc.vector.tensor_copy(hT16[:], hT32[:])
                psw = pm2.tile([128, 512], F32, tag="ps_w2h")
                for uo in range(KU):
                    for ui in range(KU):
                        nc.tensor.matmul(
                            psw[:, uo * 128:(uo + 1) * 128],
                            w2_sb[:, ui * U + uo * 128: ui * U + (uo + 1) * 128],
                            hT16[:, ui * 128:(ui + 1) * 128],
                            start=(ui == 0), stop=(ui == KU - 1))
                nc.vector.tensor_copy(w2h_sb[:], psw[:])
                # tanh(h)^T for the W3 matmul tail
                nc.scalar.activation(th16[:], hT32[:], AF.Tanh)

                # PE warmup during the gather/transpose window (keeps HAM at
                # full clock); results are never read.
                warm = pm2.tile([128, 512], F32, tag="ps_warm")
                for i in range(56):
                    nc.tensor.matmul(warm[:], w1_sb[:, 0:128],
                                     w1_sb[:, (i % 7) * 512:(i % 7) * 512 + 512],
                                     start=True, stop=True)

            # cast to bf16 + one multi-tile xbar transpose per grid row:
            # localT[c0, (l*KC+cc)*128 + b] = local_bf[b, l*C + cc*128 + c0]
            localT3 = localT[:].rearrange("p (t b) -> p t b", b=128)
            for j in range(WIN):
                nc.vector.tensor_copy(local_bf[:, j * WIN * C:(j + 1) * WIN * C],
                                      local_nat[:, j * WIN * C:(j + 1) * WIN * C])
            for j in range(WIN):
                tr = nc.sync.dma_start(
                    localT3[:, j * WIN * KC:(j + 1) * WIN * KC, :],
                    local_bf[:, j * WIN * C:(j + 1) * WIN * C],
                    transpose=True)
                for g in gathers:
                    add_dep_helper(tr.ins, g.ins,
                                   reason="xbar transpose after all gathers")

            # ---- scoreT = tanh(W1^T localT + w2h + b12)  [u, l*128+b] ----
            localT4 = localT[:].rearrange("p (l k b) -> p l k b", k=KC, b=128)
            with tc.tile_pool(name="sps", bufs=2, space="PSUM") as sps:
                for uo in range(KU):
                    pss = [sps.tile([128, 384], F32, tag=f"ps_s{j}",
                                    name=f"pss{uo}_{j}")
                           for j in range(WIN)]
                    for k in range(KC):
                        for j in range(WIN):
                            nc.tensor.matmul(
                                pss[j][:],
                                w1_sb[:, k * U + uo * 128:k * U + (uo + 1) * 128],
                                localT4[:, 3 * j:3 * j + 3, k, :],
                                start=(k == 0), stop=(k == KC - 1))
                    for j in range(WIN):
                        nc.vector.tensor_tensor(
                            stmp[:, j * 384:(j + 1) * 384]
                                .rearrange("p (l b) -> p l b", b=128),
                            pss[j][:].rearrange("p (l b) -> p l b", b=128),
                            w2h_sb[:, uo * 128:(uo + 1) * 128].unsqueeze(1)
                                .broadcast_to([128, WIN, 128]),
                            ALU.add)
                    nc.scalar.activation(scoreT[:, uo * RPW:(uo + 1) * RPW], stmp[:],
                                         AF.Tanh, bias=b12_sb[:, uo:uo + 1])

            # ---- logits -> softmax -> attn ----
            with tc.tile_pool(name="lgps", bufs=1, space="PSUM") as lgps:
                lg = lgps.tile([128, NL], F32, tag="ps_lg")
                for l in range(NL):
                    for uo in range(KU):
                        nc.tensor.matmul(
                            lg[:, l:l + 1],
                            scoreT[:, uo * RPW + l * 128:uo * RPW + (l + 1) * 128],
                            v1_sb[:, uo:uo + 1],
                            start=(uo == 0), stop=(uo == KU - 1))
                nc.vector.tensor_reduce(negmax[:], lg[:], axis=mybir.AxisListType.X,
                                        op=ALU.max, negate=True)
                nc.scalar.activation(e_sb[:], lg[:], AF.Exp, bias=negmax[:])
            nc.vector.reduce_sum(esum[:], e_sb[:], axis=mybir.AxisListType.X)
            nc.vector.reciprocal(rsum[:], esum[:])
            nc.vector.scalar_tensor_tensor(attn_f[:], e_sb[:], rsum[:], gauss_sb,
                                           ALU.mult, ALU.mult)
            nc.sync.dma_start(attn[:], attn_f[:])

            # ---- ctx^T via diag matmuls, then tanh -> tct16 ----
            for l in range(NL):
                nc.vector.tensor_scalar_mul(diag[:, l * 128:(l + 1) * 128], eye_sb,
                                            attn_f[:, l:l + 1])
            with tc.tile_pool(name="cps", bufs=4, space="PSUM") as cps, \
                 tc.tile_pool(name="ops", bufs=1, space="PSUM") as ops:
                for cc in range(KC):
                    pc = cps.tile([128, 128], F32, tag="ps_c", name=f"pc{cc}")
                    for l in range(NL):
                        nc.tensor.matmul(pc[:],
                                         local_bf[:, l * C + cc * 128:l * C + (cc + 1) * 128],
                                         diag[:, l * 128:(l + 1) * 128],
                                         start=(l == 0), stop=(l == NL - 1))
                    nc.scalar.activation(tct16[:, cc * 128:(cc + 1) * 128], pc[:], AF.Tanh)

                # ---- out = tanh([ctx, h]) @ W3 + W3_b ----
                po = ops.tile([128, U], F32, tag="ps_o")
                for kk in range(KT):
                    lhsT = (tct16[:, kk * 128:(kk + 1) * 128] if kk < KC
                            else th16[:, (kk - KC) * 128:(kk - KC + 1) * 128])
                    nc.tensor.matmul(po[:], lhsT, w3_sb[:, kk * U:(kk + 1) * U],
                                     start=(kk == 0), stop=(kk == KT - 1))
                nc.vector.tensor_tensor(out_sb[:], po[:], w3b_sb, ALU.add)
            nc.sync.dma_start(out[:], out_sb[:])

    nc.compile()
    return nc


_NC_CACHE = None


def _get_nc():
    global _NC_CACHE
    if _NC_CACHE is None:
        _NC_CACHE = _build_nc()
    return _NC_CACHE


def _chunked(w, k):
    """[k*128, n] -> [128, k*n] with chunk-major columns."""
    n = w.shape[1]
    return np.ascontiguousarray(
        w.reshape(k, 128, n).transpose(1, 0, 2).reshape(128, k * n))


def make_host_inputs(features, hidden, W1_w, W1_b, W2_w, W2_b, V1_w, V1_b,
                     W3_w, W3_b, Wa, Wb):
    """Build the 8 per-core input maps."""
    bf = ml_dtypes.bfloat16
    f = np.float32

    jj, kk = np.meshgrid(np.arange(WIN), np.arange(WIN), indexing="ij")
    d2 = (jj - WIN / 2.0) ** 2 + (kk - WIN / 2.0) ** 2
    gauss_row = np.exp(-d2 / (0.5 * D * D)).reshape(NL).astype(f)

    p = np.arange(128)
    q = np.arange(8)

    w3cat = np.zeros((128, KT * U + 128 + KU), bf)
    w3cat[:, 0:KT * U] = _chunked(np.asarray(W3_w, f), KT).astype(bf)
    w3cat[:, KT * U:KT * U + 128] = np.eye(128, dtype=f).astype(bf)
    w3cat[:, KT * U + 128:] = _chunked(np.asarray(V1_w, f), KU).astype(bf)

    fcrit = np.zeros((128, FC_COLS), f)
    fcrit[:, F_ID:F_ID + 128] = np.eye(128, dtype=f)
    fcrit[:, F_WA:F_WA + KU * 100] = _chunked(np.asarray(Wa, f), KU)
    fcrit[:, F_MASK:F_MASK + 128] = (
        np.arange(128)[None, :] % 16 == p[:, None] % 16)
    fcrit[0, F_COMB:F_COMB + 128] = float(G)
    fcrit[1, F_COMB:F_COMB + 128] = 1.0

    frest = np.zeros((128, FR_COLS), f)
    frest[:, F_W3B:F_GAUSS] = np.broadcast_to(np.asarray(W3_b, f), (128, U))
    frest[:, F_GAUSS:F_IOTA] = np.broadcast_to(gauss_row, (128, NL))
    frest[:, F_IOTA:F_B12] = L * (q[None, :] * 16 + (p[:, None] % 16))
    frest[:, F_B12:F_B12 + KU] = _chunked(
        (np.asarray(W1_b, f) + np.asarray(W2_b, f)).reshape(U, 1), KU)

    shared = {
        "fcrit": fcrit,
        "frest": frest,
        "w1cat": _chunked(np.asarray(W1_w, f), KC).astype(bf),
        "w2cat": _chunked(np.asarray(W2_w, f), KU).astype(bf),
        "w3cat": w3cat,
        "wbd": np.ascontiguousarray(Wb, f),
    }
    features = np.asarray(features, f)
    hidden = np.asarray(hidden, f)
    in_maps = []
    for c in range(NC_CORES):
        sl = slice(c * BS, (c + 1) * BS)
        m = dict(shared)
        m["feat"] = np.ascontiguousarray(features[sl]).reshape(NROWS, C)
        m["hid"] = np.ascontiguousarray(hidden[sl])
        in_maps.append(m)
    return in_maps


def kernel(features, hidden, W1_w, W1_b, W2_w, W2_b, V1_w, V1_b,
           W3_w, W3_b, Wa, Wb, _run_kwargs=None):
    nc = _get_nc()
    in_maps = make_host_inputs(features, hidden, W1_w, W1_b, W2_w, W2_b,
                               V1_w, V1_b, W3_w, W3_b, Wa, Wb)
    res = run_bass_kernel_spmd(nc, in_maps, core_ids=list(range(NC_CORES)),
                               **(_run_kwargs or {}))
    out = np.concatenate([r["out"] for r in res.results], axis=0)
    attn = np.concatenate([r["attn"] for r in res.results], axis=0)
    kernel.last_results = res
    return out, attn.reshape(B, NL, 1)
